# revision 15
# baseline (speedup 1.0000x reference)
"""Trainium2 Bass kernel for nn_Model_1778116460915 (gnn_message_passing).

Per-node MLP stack (ChebConv K=1 does no propagation; edge data unused):
  h = x @ We + be; x_embed = h; 3x {temporal convs, cheb, FF, layernorms,
  gated fusion}; out = h @ Wo + bo.  Returns (out, x_embed).

Strategy:
  - Nodes sharded across 8 cores (12544/core, N padded to 100352).
  - Feature-major on-chip layout: [128 partitions, nodes] tiles = 2 node
    groups x 64 features; all matmuls are 128x128 block-diagonal float32r
    (FP22) at full PE rate.
  - LayerNorm: mean-centering folded into weights (C = I - J/64), variance
    via squares + ones-matmul, rsqrt via integer magic-constant seed
    (int32<->fp32 ACT port conversion) + 2 Newton steps; the rsqrt chain is
    shared across groups of 4 chunks (stats DMA-gathered into partition
    rows 0/32/64/96 of one tile) to amortize per-op cost.
  - eps folding: stats weights pre-scaled by 1/64; the reference's +eps in
    rsqrt(var+eps) is dropped (relative effect <= ~1e-4 for this model's
    variance ranges, far below the fp22 matmul noise).
  - Elementwise work balanced across ACT / DVE / GPSIMD engines.
"""
import sys
import numpy as np

for _p in ('/opt/trn_rl_repo', '/root/.axon_site/_ro/trn_rl_repo'):
    if _p not in sys.path:
        sys.path.append(_p)

N = 100000
F_IN = 16
D = 64
FF = 256
OUT_LEN = 12
L = 3
EPS = 1e-5

NCORES = 8
CN = 12544              # nodes per core (padded)
NPAD = NCORES * CN      # 100352
G = CN // 2             # free-dim length per core (2 groups packed) = 6272
CHUNKS = [512] * 11 + [320, 320]       # sum = 6272; each >=256 (f32r rate)
GROUPS = [[0, 1, 2, 3], [4, 5, 6, 7], [8, 9, 10, 11], [12]]
MAGIC = 1597463007.0    # 0x5F3759DF
NR_ITERS = 2
CFG = {"mm": 4, "acc": 2, "stat": 1, "bc": 1, "t3": 3, "t5": 5}

_COMPILED = {}
TRACE = False
LAST_EXEC_NS = None
LAST_RESULTS = None


def _slot_layout(flags):
    wslots = ["CBD", "SBD", "BBD", "IBD", "IBDN", "EMB", "OUT"]
    bslots = ["MAGIC", "BOUT"]
    for i in range(L):
        wslots += [f"P{i}", f"Q{i}", f"R{i}", f"CH{i}",
                   f"P2{i}", f"Q2{i}", f"R2{i}"]
        wslots += [f"F1{i}_{j}" for j in range(4)]
        wslots += [f"F2{i}_{j}" for j in range(4)]
        wslots += [f"FS{i}", f"FGQ{i}", f"FGO{i}"]
        bslots += [f"bP{i}", f"bQ{i}", f"bR{i}", f"bC{i}",
                   f"bP2{i}", f"bQ2{i}", f"bR2{i}"]
        bslots += [f"b1{i}_{j}" for j in range(4)]
        bslots += [f"cb2{i}", f"bFG{i}", f"bFS{i}",
                   f"g1{i}", f"h1{i}", f"g2{i}", f"h2{i}",
                   f"gB{i}", f"hB{i}"]
    wi = {n: k for k, n in enumerate(wslots)}
    bi = {n: k for k, n in enumerate(bslots)}
    return wslots, bslots, wi, bi


def _bd(a):
    z = np.zeros((128, 128), np.float32)
    z[0:64, 0:64] = a
    z[64:128, 64:128] = a
    return z


def _dup(v):
    return np.concatenate([v, v]).astype(np.float32)


def _trivial(g, b):
    return bool(np.all(g == 1.0) and np.all(b == 0.0))


def _make_flags(p):
    ln1t = [_trivial(p["ln1_g"][i], p["ln1_b"][i]) for i in range(L)]
    ln2t = [_trivial(p["ln2_g"][i], p["ln2_b"][i]) for i in range(L)]
    blkt = [_trivial(p["blk_g"][i], p["blk_b"][i]) for i in range(L)]
    ln1_full = tuple(
        (i == 0) or (not ln1t[i]) or (not blkt[i - 1]) for i in range(L))
    return ln1_full, tuple(ln2t), tuple(blkt), tuple(ln1t)


def _pack_host(p, flags):
    wslots, bslots, wi, bi = _slot_layout(flags)
    C = np.eye(64, dtype=np.float32) - np.float32(1.0 / 64.0)
    W = np.zeros((128, len(wslots) * 128), np.float32)
    B = np.zeros((128, len(bslots)), np.float32)

    def setw(name, m):
        k = wi[name]
        W[:, k * 128:(k + 1) * 128] = m

    def setb(name, v):
        B[:, bi[name]] = v

    setw("CBD", _bd(C))
    sbd = np.zeros((128, 128), np.float32)
    sbd[0:64, 0] = 1.0 / 64.0          # stats matmul yields variance directly
    sbd[64:128, 1] = 1.0 / 64.0
    setw("SBD", sbd)
    bbd = np.zeros((128, 128), np.float32)
    for k in range(4):                  # replicated for quad rows 0/32/64/96
        bbd[32 * k, 0:64] = 1.0
        bbd[32 * k + 1, 64:128] = 1.0
    setw("BBD", bbd)
    setw("IBD", np.eye(128, dtype=np.float32))
    setw("IBDN", -np.eye(128, dtype=np.float32))
    emb = np.zeros((128, 128), np.float32)
    emb[0:16, 0:64] = p["embed_w"]
    emb[16:32, 64:128] = p["embed_w"]
    emb[32, 0:64] = p["embed_b"]
    emb[32, 64:128] = p["embed_b"]
    setw("EMB", emb)
    ow = np.zeros((128, 128), np.float32)
    ow[0:64, 0:12] = p["out_w"]
    ow[64:128, 12:24] = p["out_w"]
    setw("OUT", ow)

    B[:, bi["MAGIC"]] = MAGIC
    bo = np.zeros(128, np.float32)
    bo[0:12] = p["out_b"]
    bo[12:24] = p["out_b"]
    setb("BOUT", bo)

    for i in range(L):
        setw(f"P{i}", _bd(p["tc1_w"][i, 0]))
        setw(f"Q{i}", _bd(p["tc1_w"][i, 1]))
        setw(f"R{i}", _bd(p["tc1_w"][i, 2]))
        setw(f"CH{i}", _bd(p["cheb_w"][i]))
        setw(f"P2{i}", _bd(p["tc2_w"][i, 0]))
        setw(f"Q2{i}", _bd(p["tc2_w"][i, 1]))
        setw(f"R2{i}", _bd(p["tc2_w"][i, 2]))
        for j in range(4):
            setw(f"F1{i}_{j}", _bd(p["ff_w1"][i][:, 64 * j:64 * (j + 1)]))
        W2C = (p["ff_w2"][i] @ C).astype(np.float32)
        for j in range(4):
            setw(f"F2{i}_{j}", _bd(W2C[64 * j:64 * (j + 1), :]))
        setw(f"FS{i}", _bd(p["fs_w"][i]))
        setw(f"FGQ{i}", _bd(p["fg_w"][i][0:64, :]))
        setw(f"FGO{i}", _bd(p["fg_w"][i][64:128, :]))

        setb(f"bP{i}", _dup(p["tc1_b"][i, 0]))
        setb(f"bQ{i}", _dup(p["tc1_b"][i, 1]))
        setb(f"bR{i}", _dup(p["tc1_b"][i, 2]))
        setb(f"bC{i}", _dup(p["cheb_b"][i]))
        setb(f"bP2{i}", _dup(p["tc2_b"][i, 0]))
        setb(f"bQ2{i}", _dup(p["tc2_b"][i, 1]))
        setb(f"bR2{i}", _dup(p["tc2_b"][i, 2]))
        for j in range(4):
            setb(f"b1{i}_{j}", _dup(p["ff_b1"][i][64 * j:64 * (j + 1)]))
        setb(f"cb2{i}", _dup(C @ p["ff_b2"][i]))
        setb(f"bFG{i}", _dup(p["fg_b"][i]))
        setb(f"bFS{i}", _dup(p["fs_b"][i]))
        setb(f"g1{i}", _dup(p["ln1_g"][i]))
        setb(f"h1{i}", _dup(p["ln1_b"][i]))
        setb(f"g2{i}", _dup(p["ln2_g"][i]))
        setb(f"h2{i}", _dup(p["ln2_b"][i]))
        setb(f"gB{i}", _dup(p["blk_g"][i]))
        setb(f"hB{i}", _dup(p["blk_b"][i]))
    return W, B


def _split_waits(nc, mybir):
    """walrus here encodes at most ONE semaphore wait per instruction; move
    extras onto engine-matched NoOps inserted right before the instruction."""
    n = 0
    for f in nc.m.functions:
        for blk in f.blocks:
            out = []
            changed = False
            for inst in blk.instructions:
                si = inst.sync_info
                waits = list(si.on_wait) if si is not None and si.on_wait else []
                if len(waits) > 1:
                    for w in waits[:-1]:
                        nop = mybir.InstNoOp(
                            name=f"{inst.name}-wsplit{n}", ins=[], outs=[])
                        nop.engine = inst.engine
                        nop.sync_info = mybir.SyncInfo(on_wait=[w], on_update=[])
                        out.append(nop)
                        n += 1
                    si.on_wait = [waits[-1]]
                    inst.sync_info = si
                    changed = True
                out.append(inst)
            if changed:
                blk.instructions = out
    return n


def _build(flags):
    import contextlib
    import concourse.bass as bass
    import concourse.mybir as mybir
    import concourse.tile as tile

    ln1_full, ln2t, blkt, ln1t = flags
    wslots, bslots, wi, bi = _slot_layout(flags)
    NW = len(wslots)
    NB = len(bslots)

    F32 = mybir.dt.float32
    F32R = mybir.dt.float32r
    I32 = mybir.dt.int32
    AF = mybir.ActivationFunctionType
    Alu = mybir.AluOpType

    nc = bass.Bass("TRN2", target_bir_lowering=False, debug=False,
                   num_devices=NCORES)
    d_x = nc.dram_tensor("x_packed", [33, G], F32R, kind="ExternalInput")
    d_w = nc.dram_tensor("wstack", [128, NW * 128], F32R, kind="ExternalInput")
    d_b = nc.dram_tensor("bstack", [128, NB], F32, kind="ExternalInput")
    d_y = nc.dram_tensor("y_out", [24, G], F32, kind="ExternalOutput")
    d_xe = nc.dram_tensor("xe_out", [128, G], F32, kind="ExternalOutput")

    with tile.TileContext(nc) as tc, contextlib.ExitStack() as ctx:
        wpool = ctx.enter_context(tc.tile_pool(name="wpool", bufs=1))
        io = ctx.enter_context(tc.tile_pool(name="io", bufs=3))
        t2p = ctx.enter_context(tc.tile_pool(name="t2p", bufs=2))
        t3p = ctx.enter_context(tc.tile_pool(name="t3p", bufs=3))
        t5p = ctx.enter_context(tc.tile_pool(name="t5p", bufs=5))
        qp = ctx.enter_context(tc.tile_pool(name="qp", bufs=2))
        ps = ctx.enter_context(tc.tile_pool(name="ps", bufs=2, space="PSUM"))

        WS = wpool.tile([128, NW * 128], F32R)
        nc.sync.dma_start(out=WS, in_=d_w.ap())
        BS = wpool.tile([128, NB], F32)
        nc.sync.dma_start(out=BS, in_=d_b.ap())

        def w(name, kslice=None, mslice=None):
            k = wi[name]
            a = WS[:, k * 128:(k + 1) * 128]
            if kslice is not None:
                a = a[kslice[0]:kslice[1]]
            if mslice is not None:
                a = a[:, 0:mslice]
            return a

        def b(name, parts=128):
            return BS[0:parts, bi[name]:bi[name] + 1]

        def mm(out, name, rhs, start, stop, kslice=None, mslice=None,
               tile_position=None):
            nc.tensor.matmul(out, w(name, kslice, mslice), rhs,
                             start=start, stop=stop,
                             tile_position=tile_position)

        FCS = [CHUNKS[c] for c in range(len(CHUNKS))]

        def ln_stats(zp, Fc, cbcol, quadv, k):
            """Evacuate centered zp, square, reduce -> variance row-pair k of
            quadv. Returns z0s (SBUF, centered values incl. bias)."""
            z0s = t5p.tile([128, 512], F32, tag="z0s")
            if cbcol is not None:
                nc.scalar.activation(z0s[:, :Fc], zp[:, :Fc], AF.Identity,
                                     bias=b(cbcol), scale=1.0)
            else:
                nc.scalar.copy(z0s[:, :Fc], zp[:, :Fc])
            sq = t3p.tile([128, 512], F32R, tag="sq")
            nc.gpsimd.tensor_tensor(out=sq[:, :Fc], in0=z0s[:, :Fc],
                                    in1=z0s[:, :Fc], op=Alu.mult)
            st = ps.tile([2, 512], F32, tag="stat", bufs=CFG["stat"])
            nc.tensor.matmul(st[:, :Fc], w("SBD", mslice=2), sq[:, :Fc],
                             start=True, stop=True)
            if k == 0:
                nc.scalar.copy(quadv[0:2, :Fc], st[:, :Fc])
            else:
                vt = t3p.tile([2, 512], F32, tag="vt")
                nc.scalar.copy(vt[:, :Fc], st[:, :Fc])
                nc.sync.dma_start(out=quadv[32 * k:32 * k + 2, :Fc],
                                  in_=vt[:, :Fc])
            return z0s

        def ln_rsqrt(quadv, top):
            """Shared seed + Newton over quad rows; returns y tile (F32R)."""
            seed = qp.tile([128, 512], I32, tag="qs")
            nc.scalar.activation(seed[0:top], quadv[0:top].bitcast(I32),
                                 AF.Identity, bias=b("MAGIC", top), scale=-0.5)
            y = seed.bitcast(F32)
            for it in range(NR_ITERS):
                last = it == NR_ITERS - 1
                a1 = qp.tile([128, 512], F32, tag="qa")
                nc.vector.tensor_tensor(out=a1[0:top], in0=y[0:top],
                                        in1=y[0:top], op=Alu.mult)
                b1 = qp.tile([128, 512], F32, tag="qb")
                nc.vector.scalar_tensor_tensor(
                    out=b1[0:top], in0=a1[0:top], scalar=-0.5,
                    in1=quadv[0:top], op0=Alu.mult, op1=Alu.mult)
                y2 = qp.tile([128, 512], F32R if last else F32, tag="qy")
                nc.vector.scalar_tensor_tensor(
                    out=y2[0:top], in0=b1[0:top], scalar=1.5,
                    in1=y[0:top], op0=Alu.add, op1=Alu.mult)
                y = y2
            return y

        def ln_apply(z0s, yq, k, Fc, out_tile, gcol, hcol, trivial_affine):
            rb = ps.tile([128, 512], F32, tag="bc", bufs=CFG["bc"])
            tp = (96, 0) if k == 3 else None
            nc.tensor.matmul(rb[:, :Fc],
                             w("BBD")[32 * k:32 * k + 2],
                             yq[32 * k:32 * k + 2, :Fc],
                             start=True, stop=True, tile_position=tp)
            if trivial_affine:
                nc.vector.tensor_tensor(out=out_tile[:, :Fc],
                                        in0=z0s[:, :Fc], in1=rb[:, :Fc],
                                        op=Alu.mult)
            else:
                pre = t3p.tile([128, 512], F32, tag="lnpre")
                nc.vector.tensor_tensor(out=pre[:, :Fc], in0=z0s[:, :Fc],
                                        in1=rb[:, :Fc], op=Alu.mult)
                nc.vector.tensor_scalar(
                    out=out_tile[:, :Fc], in0=pre[:, :Fc],
                    scalar1=b(gcol), scalar2=b(hcol),
                    op0=Alu.mult, op1=Alu.add)

        # ================= main loop: groups of chunks =================
        for group in GROUPS:
            top = 32 * (len(group) - 1) + 2
            st = {}
            # ---- embed ----
            for k, c in enumerate(group):
                Fc = FCS[c]
                sl = slice(sum(FCS[:c]), sum(FCS[:c]) + Fc)
                xt = io.tile([33, 512], F32R, tag="xt")
                nc.sync.dma_start(out=xt[:, :Fc], in_=d_x.ap()[:, sl])
                ep = ps.tile([128, 512], F32, tag="acc", bufs=CFG["acc"])
                mm(ep[:, :Fc], "EMB", xt[:, :Fc], True, True, kslice=(0, 33))
                h = t5p.tile([128, 512], F32R, tag="h0")
                nc.scalar.copy(h[:, :Fc], ep[:, :Fc])
                nc.sync.dma_start(out=d_xe.ap()[:, sl],
                                  in_=h[:, :Fc].bitcast(F32))
                st[c] = {"h": h, "sl": sl, "Fc": Fc}

            for i in range(L):
                # ---- LN1 (full only when needed) ----
                if ln1_full[i]:
                    qv1 = qp.tile([128, 512], F32, tag="qv")
                    for k, c in enumerate(group):
                        Fc = st[c]["Fc"]
                        z1 = ps.tile([128, 512], F32, tag="acc", bufs=CFG["acc"])
                        mm(z1[:, :Fc], "CBD", st[c]["h"][:, :Fc], True, True)
                        st[c]["z1s"] = ln_stats(z1, Fc, None, qv1, k)
                    yq1 = ln_rsqrt(qv1, top)
                    for k, c in enumerate(group):
                        Fc = st[c]["Fc"]
                        xn = t5p.tile([128, 512], F32R, tag="xn")
                        ln_apply(st[c]["z1s"], yq1, k, Fc, xn,
                                 f"g1{i}", f"h1{i}", ln1t[i])
                        st[c]["xn"] = xn
                else:
                    for c in group:
                        st[c]["xn"] = st[c]["h"]

                # ---- front: tc1, cheb, tc2, ff -> LN2 stats ----
                qv2 = qp.tile([128, 512], F32, tag="qv")
                for k, c in enumerate(group):
                    Fc = st[c]["Fc"]
                    q = st[c]["h"]
                    xn = st[c]["xn"]
                    Pp = ps.tile([128, 512], F32, tag="mm", bufs=CFG["mm"])
                    mm(Pp[:, :Fc], f"P{i}", q[:, :Fc], True, True)
                    Qp = ps.tile([128, 512], F32, tag="mm", bufs=CFG["mm"])
                    mm(Qp[:, :Fc], f"Q{i}", q[:, :Fc], True, True)
                    sQ = t3p.tile([128, 512], F32, tag="sig")
                    nc.scalar.activation(sQ[:, :Fc], Qp[:, :Fc], AF.Sigmoid,
                                         bias=b(f"bQ{i}"), scale=1.0)
                    u = t3p.tile([128, 512], F32R, tag="u")
                    nc.vector.scalar_tensor_tensor(
                        out=u[:, :Fc], in0=Pp[:, :Fc], scalar=b(f"bP{i}"),
                        in1=sQ[:, :Fc], op0=Alu.add, op1=Alu.mult)
                    Rp = ps.tile([128, 512], F32, tag="mm", bufs=CFG["mm"])
                    mm(Rp[:, :Fc], f"R{i}", q[:, :Fc], True, False)
                    mm(Rp[:, :Fc], "IBD", u[:, :Fc], False, True)
                    t = t3p.tile([128, 512], F32R, tag="t")
                    nc.scalar.activation(t[:, :Fc], Rp[:, :Fc], AF.Relu,
                                         bias=b(f"bR{i}"), scale=1.0)

                    Cp = ps.tile([128, 512], F32, tag="mm", bufs=CFG["mm"])
                    mm(Cp[:, :Fc], f"CH{i}", t[:, :Fc], True, True)
                    tt = t3p.tile([128, 512], F32R, tag="t")
                    nc.scalar.activation(tt[:, :Fc], Cp[:, :Fc], AF.Relu,
                                         bias=b(f"bC{i}"), scale=1.0)

                    P2p = ps.tile([128, 512], F32, tag="mm", bufs=CFG["mm"])
                    mm(P2p[:, :Fc], f"P2{i}", tt[:, :Fc], True, True)
                    Q2p = ps.tile([128, 512], F32, tag="mm", bufs=CFG["mm"])
                    mm(Q2p[:, :Fc], f"Q2{i}", tt[:, :Fc], True, True)
                    sQ2 = t3p.tile([128, 512], F32, tag="sig")
                    nc.scalar.activation(sQ2[:, :Fc], Q2p[:, :Fc], AF.Sigmoid,
                                         bias=b(f"bQ2{i}"), scale=1.0)
                    u2 = t3p.tile([128, 512], F32R, tag="u")
                    nc.vector.scalar_tensor_tensor(
                        out=u2[:, :Fc], in0=P2p[:, :Fc], scalar=b(f"bP2{i}"),
                        in1=sQ2[:, :Fc], op0=Alu.add, op1=Alu.mult)
                    R2p = ps.tile([128, 512], F32, tag="mm", bufs=CFG["mm"])
                    mm(R2p[:, :Fc], f"R2{i}", tt[:, :Fc], True, False)
                    mm(R2p[:, :Fc], "IBD", u2[:, :Fc], False, True)
                    o = t5p.tile([128, 512], F32R, tag="o")
                    nc.scalar.activation(o[:, :Fc], R2p[:, :Fc], AF.Relu,
                                         bias=b(f"bR2{i}"), scale=1.0)
                    st[c]["o"] = o

                    zp = ps.tile([128, 512], F32, tag="acc", bufs=CFG["acc"])
                    mm(zp[:, :Fc], "CBD", xn[:, :Fc], True, False)
                    for j in range(4):
                        fp = ps.tile([128, 512], F32, tag="mm", bufs=CFG["mm"])
                        mm(fp[:, :Fc], f"F1{i}_{j}", xn[:, :Fc], True, True)
                        fj = t3p.tile([128, 512], F32R, tag="fj")
                        if j % 2 == 0:
                            nc.scalar.activation(
                                fj[:, :Fc], fp[:, :Fc], AF.Relu,
                                bias=b(f"b1{i}_{j}"), scale=1.0)
                        else:
                            nc.vector.tensor_scalar(
                                out=fj[:, :Fc], in0=fp[:, :Fc],
                                scalar1=b(f"b1{i}_{j}"), scalar2=0.0,
                                op0=Alu.add, op1=Alu.max)
                        mm(zp[:, :Fc], f"F2{i}_{j}", fj[:, :Fc],
                           False, j == 3)
                    st[c]["z2s"] = ln_stats(zp, Fc, f"cb2{i}", qv2, k)

                yq2 = ln_rsqrt(qv2, top)

                # ---- mid: us, gating, blk stats ----
                qvb = qp.tile([128, 512], F32, tag="qv")
                for k, c in enumerate(group):
                    Fc = st[c]["Fc"]
                    q = st[c]["h"]
                    o = st[c]["o"]
                    us = t3p.tile([128, 512], F32R, tag="us")
                    ln_apply(st[c]["z2s"], yq2, k, Fc, us,
                             f"g2{i}", f"h2{i}", ln2t[i])
                    fgp = ps.tile([128, 512], F32, tag="mm", bufs=CFG["mm"])
                    mm(fgp[:, :Fc], f"FGQ{i}", q[:, :Fc], True, False)
                    mm(fgp[:, :Fc], f"FGO{i}", o[:, :Fc], False, True)
                    fgx = t3p.tile([128, 512], F32R, tag="fgx")
                    nc.scalar.activation(fgx[:, :Fc], fgp[:, :Fc],
                                         AF.Identity, bias=b(f"bFG{i}"),
                                         scale=1.0)
                    fsp = ps.tile([128, 512], F32, tag="mm", bufs=CFG["mm"])
                    mm(fsp[:, :Fc], "IBD", fgx[:, :Fc], True, False)
                    mm(fsp[:, :Fc], f"FS{i}", us[:, :Fc], False, True)
                    g = t3p.tile([128, 512], F32, tag="g")
                    nc.scalar.activation(g[:, :Fc], fsp[:, :Fc], AF.Sigmoid,
                                         bias=b(f"bFS{i}"), scale=1.0)
                    dps = ps.tile([128, 512], F32, tag="mm", bufs=CFG["mm"])
                    mm(dps[:, :Fc], "IBD", us[:, :Fc], True, False)
                    mm(dps[:, :Fc], "IBDN", fgx[:, :Fc], False, True)
                    e = t3p.tile([128, 512], F32R, tag="e")
                    nc.vector.tensor_tensor(out=e[:, :Fc], in0=g[:, :Fc],
                                            in1=dps[:, :Fc], op=Alu.mult)
                    zb = ps.tile([128, 512], F32, tag="acc", bufs=CFG["acc"])
                    mm(zb[:, :Fc], "CBD", fgx[:, :Fc], True, False)
                    mm(zb[:, :Fc], "CBD", q[:, :Fc], False, False)
                    mm(zb[:, :Fc], "CBD", e[:, :Fc], False, True)
                    st[c]["zbs"] = ln_stats(zb, Fc, None, qvb, k)

                yqb = ln_rsqrt(qvb, top)

                # ---- tail: normalize into next h ----
                for k, c in enumerate(group):
                    Fc = st[c]["Fc"]
                    hn = t5p.tile([128, 512], F32R, tag=f"h{i + 1}")
                    ln_apply(st[c]["zbs"], yqb, k, Fc, hn,
                             f"gB{i}", f"hB{i}", blkt[i])
                    st[c]["h"] = hn

            # ---- output projection ----
            for k, c in enumerate(group):
                Fc = st[c]["Fc"]
                yp = ps.tile([24, 512], F32, tag="acc", bufs=CFG["acc"])
                mm(yp[:, :Fc], "OUT", st[c]["h"][:, :Fc], True, True,
                   mslice=24)
                yt = io.tile([24, 512], F32, tag="yt")
                nc.scalar.activation(yt[:, :Fc], yp[:, :Fc], AF.Identity,
                                     bias=b("BOUT", 24), scale=1.0)
                nc.sync.dma_start(out=d_y.ap()[:, st[c]["sl"]],
                                  in_=yt[:, :Fc])

    _split_waits(nc, mybir)
    return nc


def _get_compiled(flags):
    if flags not in _COMPILED:
        _COMPILED[flags] = _build(flags)
    return _COMPILED[flags]


def kernel(x, edge_index, edge_weight, params):
    from concourse import bass_utils

    x = np.asarray(x, dtype=np.float32)
    p = {k: np.asarray(v, dtype=np.float32) for k, v in params.items()}

    flags = _make_flags(p)
    nc = _get_compiled(flags)

    W, B = _pack_host(p, flags)

    xpad = np.zeros((NPAD, F_IN), np.float32)
    xpad[:N] = x
    xc = xpad.reshape(NCORES, CN, F_IN)
    in_maps = []
    for c in range(NCORES):
        xd = np.ones((33, G), np.float32)
        xd[0:16] = xc[c, :G].T
        xd[16:32] = xc[c, G:].T
        in_maps.append({"x_packed": xd, "wstack": W, "bstack": B})

    res = bass_utils.run_bass_kernel_spmd(
        nc, in_maps, core_ids=list(range(NCORES)), trace=TRACE)
    global LAST_EXEC_NS, LAST_RESULTS
    LAST_EXEC_NS = res.exec_time_ns
    LAST_RESULTS = res

    out = np.zeros((NPAD, OUT_LEN), np.float32)
    xe = np.zeros((NPAD, D), np.float32)
    for c in range(NCORES):
        y = res.results[c]["y_out"]
        e = res.results[c]["xe_out"]
        base = c * CN
        out[base:base + G] = y[0:12].T
        out[base + G:base + CN] = y[12:24].T
        xe[base:base + G] = e[0:64].T
        xe[base + G:base + CN] = e[64:128].T
    return out[:N], xe[:N]


# revision 20
# speedup vs baseline: 1.0439x; 1.0439x over previous
"""Trainium2 Bass kernel for nn_Model_1778116460915 (gnn_message_passing).

Per-node MLP stack (ChebConv K=1 does no propagation; edge data unused):
  h = x @ We + be; x_embed = h; 3x {temporal convs, cheb, FF, layernorms,
  gated fusion}; out = h @ Wo + bo.  Returns (out, x_embed).

Strategy:
  - Nodes sharded across 8 cores (12544/core, N padded to 100352).
  - Feature-major on-chip layout: [128 partitions, nodes] tiles = 2 node
    groups x 64 features; all matmuls are 128x128 block-diagonal float32r
    (FP22) at full PE rate.
  - LayerNorm: mean-centering folded into weights (C = I - J/64), variance
    via squares + ones-matmul, rsqrt via integer magic-constant seed
    (int32<->fp32 ACT port conversion) + 2 Newton steps; the rsqrt chain is
    shared across groups of 4 chunks (stats DMA-gathered into partition
    rows 0/32/64/96 of one tile) to amortize per-op cost.
  - eps folding: stats weights pre-scaled by 1/64; the reference's +eps in
    rsqrt(var+eps) is dropped (relative effect <= ~1e-4 for this model's
    variance ranges, far below the fp22 matmul noise).
  - Elementwise work balanced across ACT / DVE / GPSIMD engines.
"""
import sys
import numpy as np

for _p in ('/opt/trn_rl_repo', '/root/.axon_site/_ro/trn_rl_repo'):
    if _p not in sys.path:
        sys.path.append(_p)

N = 100000
F_IN = 16
D = 64
FF = 256
OUT_LEN = 12
L = 3
EPS = 1e-5

NCORES = 8
CN = 12544              # nodes per core (padded)
NPAD = NCORES * CN      # 100352
G = CN // 2             # free-dim length per core (2 groups packed) = 6272
CHUNKS = [512] * 11 + [320, 320]       # sum = 6272; each >=256 (f32r rate)
GROUPS = [[0, 1, 2, 3], [4, 5, 6, 7], [8, 9, 10, 11], [12]]
MAGIC = 1597463007.0    # 0x5F3759DF
NR_ITERS = 2
CFG = {"mm": 4, "acc": 2, "stat": 1, "bc": 1, "t3": 4, "t5": 5, "h": 10,
       "e_t": "act", "e_t2": "act", "e_o": "act", "ff_act": 2,
       "e_fgx": "act", "e_z0s": "act", "e_stat": "dve", "e_sq": "act"}

_COMPILED = {}
TRACE = False
LAST_EXEC_NS = None
LAST_RESULTS = None


def _slot_layout(flags):
    wslots = ["CBD", "SBD", "BBD", "IBD", "IBDN", "EMB", "OUT"]
    bslots = ["MAGIC", "BOUT"]
    for i in range(L):
        wslots += [f"P{i}", f"Q{i}", f"R{i}", f"CH{i}",
                   f"P2{i}", f"Q2{i}", f"R2{i}"]
        wslots += [f"F1{i}_{j}" for j in range(4)]
        wslots += [f"F2{i}_{j}" for j in range(4)]
        wslots += [f"FS{i}", f"FGQ{i}", f"FGO{i}"]
        bslots += [f"bP{i}", f"bQ{i}", f"bR{i}", f"bC{i}",
                   f"bP2{i}", f"bQ2{i}", f"bR2{i}"]
        bslots += [f"b1{i}_{j}" for j in range(4)]
        bslots += [f"cb2{i}", f"bFG{i}", f"bFS{i}",
                   f"g1{i}", f"h1{i}", f"g2{i}", f"h2{i}",
                   f"gB{i}", f"hB{i}"]
    wi = {n: k for k, n in enumerate(wslots)}
    bi = {n: k for k, n in enumerate(bslots)}
    return wslots, bslots, wi, bi


def _bd(a):
    z = np.zeros((128, 128), np.float32)
    z[0:64, 0:64] = a
    z[64:128, 64:128] = a
    return z


def _dup(v):
    return np.concatenate([v, v]).astype(np.float32)


def _trivial(g, b):
    return bool(np.all(g == 1.0) and np.all(b == 0.0))


def _make_flags(p):
    ln1t = [_trivial(p["ln1_g"][i], p["ln1_b"][i]) for i in range(L)]
    ln2t = [_trivial(p["ln2_g"][i], p["ln2_b"][i]) for i in range(L)]
    blkt = [_trivial(p["blk_g"][i], p["blk_b"][i]) for i in range(L)]
    ln1_full = tuple(
        (i == 0) or (not ln1t[i]) or (not blkt[i - 1]) for i in range(L))
    return ln1_full, tuple(ln2t), tuple(blkt), tuple(ln1t)


def _pack_host(p, flags):
    wslots, bslots, wi, bi = _slot_layout(flags)
    C = np.eye(64, dtype=np.float32) - np.float32(1.0 / 64.0)
    W = np.zeros((128, len(wslots) * 128), np.float32)
    B = np.zeros((128, len(bslots)), np.float32)

    def setw(name, m):
        k = wi[name]
        W[:, k * 128:(k + 1) * 128] = m

    def setb(name, v):
        B[:, bi[name]] = v

    setw("CBD", _bd(C))
    sbd = np.zeros((128, 128), np.float32)
    sbd[0:64, 0] = 1.0 / 64.0          # stats matmul yields variance directly
    sbd[64:128, 1] = 1.0 / 64.0
    setw("SBD", sbd)
    bbd = np.zeros((128, 128), np.float32)
    for k in range(4):                  # replicated for quad rows 0/32/64/96
        bbd[32 * k, 0:64] = 1.0
        bbd[32 * k + 1, 64:128] = 1.0
    setw("BBD", bbd)
    setw("IBD", np.eye(128, dtype=np.float32))
    setw("IBDN", -np.eye(128, dtype=np.float32))
    emb = np.zeros((128, 128), np.float32)
    emb[0:16, 0:64] = p["embed_w"]
    emb[16:32, 64:128] = p["embed_w"]
    emb[32, 0:64] = p["embed_b"]
    emb[32, 64:128] = p["embed_b"]
    setw("EMB", emb)
    ow = np.zeros((128, 128), np.float32)
    ow[0:64, 0:12] = p["out_w"]
    ow[64:128, 12:24] = p["out_w"]
    setw("OUT", ow)

    B[:, bi["MAGIC"]] = MAGIC
    bo = np.zeros(128, np.float32)
    bo[0:12] = p["out_b"]
    bo[12:24] = p["out_b"]
    setb("BOUT", bo)

    for i in range(L):
        setw(f"P{i}", _bd(p["tc1_w"][i, 0]))
        setw(f"Q{i}", _bd(p["tc1_w"][i, 1]))
        setw(f"R{i}", _bd(p["tc1_w"][i, 2]))
        setw(f"CH{i}", _bd(p["cheb_w"][i]))
        setw(f"P2{i}", _bd(p["tc2_w"][i, 0]))
        setw(f"Q2{i}", _bd(p["tc2_w"][i, 1]))
        setw(f"R2{i}", _bd(p["tc2_w"][i, 2]))
        for j in range(4):
            setw(f"F1{i}_{j}", _bd(p["ff_w1"][i][:, 64 * j:64 * (j + 1)]))
        W2C = (p["ff_w2"][i] @ C).astype(np.float32)
        for j in range(4):
            setw(f"F2{i}_{j}", _bd(W2C[64 * j:64 * (j + 1), :]))
        setw(f"FS{i}", _bd(p["fs_w"][i]))
        setw(f"FGQ{i}", _bd(p["fg_w"][i][0:64, :]))
        setw(f"FGO{i}", _bd(p["fg_w"][i][64:128, :]))

        setb(f"bP{i}", _dup(p["tc1_b"][i, 0]))
        setb(f"bQ{i}", _dup(p["tc1_b"][i, 1]))
        setb(f"bR{i}", _dup(p["tc1_b"][i, 2]))
        setb(f"bC{i}", _dup(p["cheb_b"][i]))
        setb(f"bP2{i}", _dup(p["tc2_b"][i, 0]))
        setb(f"bQ2{i}", _dup(p["tc2_b"][i, 1]))
        setb(f"bR2{i}", _dup(p["tc2_b"][i, 2]))
        for j in range(4):
            setb(f"b1{i}_{j}", _dup(p["ff_b1"][i][64 * j:64 * (j + 1)]))
        setb(f"cb2{i}", _dup(C @ p["ff_b2"][i]))
        setb(f"bFG{i}", _dup(p["fg_b"][i]))
        setb(f"bFS{i}", _dup(p["fs_b"][i]))
        setb(f"g1{i}", _dup(p["ln1_g"][i]))
        setb(f"h1{i}", _dup(p["ln1_b"][i]))
        setb(f"g2{i}", _dup(p["ln2_g"][i]))
        setb(f"h2{i}", _dup(p["ln2_b"][i]))
        setb(f"gB{i}", _dup(p["blk_g"][i]))
        setb(f"hB{i}", _dup(p["blk_b"][i]))
    return W, B


def _split_waits(nc, mybir):
    """walrus here encodes at most ONE semaphore wait per instruction; move
    extras onto engine-matched NoOps inserted right before the instruction."""
    n = 0
    for f in nc.m.functions:
        for blk in f.blocks:
            out = []
            changed = False
            for inst in blk.instructions:
                si = inst.sync_info
                waits = list(si.on_wait) if si is not None and si.on_wait else []
                if len(waits) > 1:
                    for w in waits[:-1]:
                        nop = mybir.InstNoOp(
                            name=f"{inst.name}-wsplit{n}", ins=[], outs=[])
                        nop.engine = inst.engine
                        nop.sync_info = mybir.SyncInfo(on_wait=[w], on_update=[])
                        out.append(nop)
                        n += 1
                    si.on_wait = [waits[-1]]
                    inst.sync_info = si
                    changed = True
                out.append(inst)
            if changed:
                blk.instructions = out
    return n


def _build(flags):
    import contextlib
    import concourse.bass as bass
    import concourse.mybir as mybir
    import concourse.tile as tile

    ln1_full, ln2t, blkt, ln1t = flags
    wslots, bslots, wi, bi = _slot_layout(flags)
    NW = len(wslots)
    NB = len(bslots)

    F32 = mybir.dt.float32
    F32R = mybir.dt.float32r
    I32 = mybir.dt.int32
    AF = mybir.ActivationFunctionType
    Alu = mybir.AluOpType

    nc = bass.Bass("TRN2", target_bir_lowering=False, debug=False,
                   num_devices=NCORES)
    d_x = nc.dram_tensor("x_packed", [33, G], F32R, kind="ExternalInput")
    d_w = nc.dram_tensor("wstack", [128, NW * 128], F32R, kind="ExternalInput")
    d_b = nc.dram_tensor("bstack", [128, NB], F32, kind="ExternalInput")
    d_y = nc.dram_tensor("y_out", [24, G], F32, kind="ExternalOutput")
    d_xe = nc.dram_tensor("xe_out", [128, G], F32, kind="ExternalOutput")

    with tile.TileContext(nc) as tc, contextlib.ExitStack() as ctx:
        wpool = ctx.enter_context(tc.tile_pool(name="wpool", bufs=1))
        io = ctx.enter_context(tc.tile_pool(name="io", bufs=3))
        t2p = ctx.enter_context(tc.tile_pool(name="t2p", bufs=2))
        t3p = ctx.enter_context(tc.tile_pool(name="t3p", bufs=CFG["t3"]))
        t5p = ctx.enter_context(tc.tile_pool(name="t5p", bufs=CFG["t5"]))
        qp = ctx.enter_context(tc.tile_pool(name="qp", bufs=2))
        ps = ctx.enter_context(tc.tile_pool(name="ps", bufs=2, space="PSUM"))

        WS = wpool.tile([128, NW * 128], F32R)
        BS = wpool.tile([128, NB], F32)
        nc.sync.dma_start(out=BS, in_=d_b.ap())
        # globals + layer-0 weights first so the embed/layer-0 pipeline can
        # start while the rest of the weight stack streams in
        w_split = 24 * 128
        nc.sync.dma_start(out=WS[:, :w_split], in_=d_w.ap()[:, :w_split])
        nc.sync.dma_start(out=WS[:, w_split:], in_=d_w.ap()[:, w_split:])

        def w(name, kslice=None, mslice=None):
            k = wi[name]
            a = WS[:, k * 128:(k + 1) * 128]
            if kslice is not None:
                a = a[kslice[0]:kslice[1]]
            if mslice is not None:
                a = a[:, 0:mslice]
            return a

        def b(name, parts=128):
            return BS[0:parts, bi[name]:bi[name] + 1]

        def mm(out, name, rhs, start, stop, kslice=None, mslice=None,
               tile_position=None):
            nc.tensor.matmul(out, w(name, kslice, mslice), rhs,
                             start=start, stop=stop,
                             tile_position=tile_position)

        def relu_ps(out_ap, ps_ap, biascol, eng):
            if eng == "act":
                nc.scalar.activation(out_ap, ps_ap, AF.Relu,
                                     bias=b(biascol), scale=1.0)
            else:
                nc.vector.tensor_scalar(
                    out=out_ap, in0=ps_ap, scalar1=b(biascol), scalar2=0.0,
                    op0=Alu.add, op1=Alu.max)

        def evac_ps(out_ap, ps_ap, biascol, eng):
            if eng == "act":
                if biascol is None:
                    nc.scalar.copy(out_ap, ps_ap)
                else:
                    nc.scalar.activation(out_ap, ps_ap, AF.Identity,
                                         bias=b(biascol), scale=1.0)
            else:
                if biascol is None:
                    nc.vector.tensor_copy(out_ap, ps_ap)
                else:
                    nc.vector.tensor_scalar(
                        out=out_ap, in0=ps_ap, scalar1=b(biascol),
                        scalar2=None, op0=Alu.add)

        FCS = [CHUNKS[c] for c in range(len(CHUNKS))]

        def ln_stats(zp, Fc, cbcol, quadv, k):
            """Evacuate centered zp, square, reduce -> variance row-pair k of
            quadv. Returns z0s (SBUF, centered values incl. bias)."""
            z0s = t5p.tile([128, 512], F32, tag="z0s")
            evac_ps(z0s[:, :Fc], zp[:, :Fc], cbcol, CFG["e_z0s"])
            sq = t3p.tile([128, 512], F32R, tag="sq")
            if CFG["e_sq"] == "act":
                nc.scalar.activation(
                    sq[:, :Fc], zp[:, :Fc], AF.Square,
                    bias=(b(cbcol) if cbcol is not None else 0.0), scale=1.0)
            else:
                nc.gpsimd.tensor_tensor(out=sq[:, :Fc], in0=z0s[:, :Fc],
                                        in1=z0s[:, :Fc], op=Alu.mult)
            st = ps.tile([2, 512], F32, tag="stat", bufs=CFG["stat"])
            nc.tensor.matmul(st[:, :Fc], w("SBD", mslice=2), sq[:, :Fc],
                             start=True, stop=True)
            if k == 0:
                evac_ps(quadv[0:2, :Fc], st[:, :Fc], None, CFG["e_stat"])
            else:
                vt = t3p.tile([2, 512], F32, tag="vt")
                evac_ps(vt[:, :Fc], st[:, :Fc], None, CFG["e_stat"])
                nc.sync.dma_start(out=quadv[32 * k:32 * k + 2, :Fc],
                                  in_=vt[:, :Fc])
            return z0s

        def ln_rsqrt(quadv, top):
            """Shared seed + Newton over quad rows; returns y tile (F32R)."""
            seed = qp.tile([128, 512], I32, tag="qs")
            nc.scalar.activation(seed[0:top], quadv[0:top].bitcast(I32),
                                 AF.Identity, bias=b("MAGIC", top), scale=-0.5)
            y = seed.bitcast(F32)
            for it in range(NR_ITERS):
                last = it == NR_ITERS - 1
                a1 = qp.tile([128, 512], F32, tag="qa")
                nc.vector.tensor_tensor(out=a1[0:top], in0=y[0:top],
                                        in1=y[0:top], op=Alu.mult)
                b1 = qp.tile([128, 512], F32, tag="qb")
                nc.vector.scalar_tensor_tensor(
                    out=b1[0:top], in0=a1[0:top], scalar=-0.5,
                    in1=quadv[0:top], op0=Alu.mult, op1=Alu.mult)
                y2 = qp.tile([128, 512], F32R if last else F32, tag="qy")
                nc.vector.scalar_tensor_tensor(
                    out=y2[0:top], in0=b1[0:top], scalar=1.5,
                    in1=y[0:top], op0=Alu.add, op1=Alu.mult)
                y = y2
            return y

        def ln_apply(z0s, yq, k, Fc, out_tile, gcol, hcol, trivial_affine):
            rb = ps.tile([128, 512], F32, tag="bc", bufs=CFG["bc"])
            tp = (96, 0) if k == 3 else None
            nc.tensor.matmul(rb[:, :Fc],
                             w("BBD")[32 * k:32 * k + 2],
                             yq[32 * k:32 * k + 2, :Fc],
                             start=True, stop=True, tile_position=tp)
            if trivial_affine:
                nc.vector.tensor_tensor(out=out_tile[:, :Fc],
                                        in0=z0s[:, :Fc], in1=rb[:, :Fc],
                                        op=Alu.mult)
            else:
                pre = t3p.tile([128, 512], F32, tag="lnpre")
                nc.vector.tensor_tensor(out=pre[:, :Fc], in0=z0s[:, :Fc],
                                        in1=rb[:, :Fc], op=Alu.mult)
                nc.vector.tensor_scalar(
                    out=out_tile[:, :Fc], in0=pre[:, :Fc],
                    scalar1=b(gcol), scalar2=b(hcol),
                    op0=Alu.mult, op1=Alu.add)

        # ================= main loop: groups of chunks =================
        for group in GROUPS:
            top = 32 * (len(group) - 1) + 2
            st = {}
            # ---- embed ----
            for k, c in enumerate(group):
                Fc = FCS[c]
                sl = slice(sum(FCS[:c]), sum(FCS[:c]) + Fc)
                xt = io.tile([33, 512], F32R, tag="xt")
                nc.sync.dma_start(out=xt[:, :Fc], in_=d_x.ap()[:, sl])
                ep = ps.tile([128, 512], F32, tag="acc", bufs=CFG["acc"])
                mm(ep[:, :Fc], "EMB", xt[:, :Fc], True, True, kslice=(0, 33))
                h = t5p.tile([128, 512], F32R, tag="h", bufs=CFG["h"])
                nc.scalar.copy(h[:, :Fc], ep[:, :Fc])
                nc.sync.dma_start(out=d_xe.ap()[:, sl],
                                  in_=h[:, :Fc].bitcast(F32))
                st[c] = {"h": h, "sl": sl, "Fc": Fc}

            for i in range(L):
                # ---- LN1 (full only when needed) ----
                if ln1_full[i]:
                    qv1 = qp.tile([128, 512], F32, tag="qv")
                    for k, c in enumerate(group):
                        Fc = st[c]["Fc"]
                        z1 = ps.tile([128, 512], F32, tag="acc", bufs=CFG["acc"])
                        mm(z1[:, :Fc], "CBD", st[c]["h"][:, :Fc], True, True)
                        st[c]["z1s"] = ln_stats(z1, Fc, None, qv1, k)
                    yq1 = ln_rsqrt(qv1, top)
                    for k, c in enumerate(group):
                        Fc = st[c]["Fc"]
                        xn = t5p.tile([128, 512], F32R, tag="xn")
                        ln_apply(st[c]["z1s"], yq1, k, Fc, xn,
                                 f"g1{i}", f"h1{i}", ln1t[i])
                        st[c]["xn"] = xn
                else:
                    for c in group:
                        st[c]["xn"] = st[c]["h"]

                # ---- front: tc1, cheb, tc2, ff -> LN2 stats ----
                qv2 = qp.tile([128, 512], F32, tag="qv")
                for k, c in enumerate(group):
                    Fc = st[c]["Fc"]
                    q = st[c]["h"]
                    xn = st[c]["xn"]
                    Pp = ps.tile([128, 512], F32, tag="mm", bufs=CFG["mm"])
                    mm(Pp[:, :Fc], f"P{i}", q[:, :Fc], True, True)
                    Qp = ps.tile([128, 512], F32, tag="mm", bufs=CFG["mm"])
                    mm(Qp[:, :Fc], f"Q{i}", q[:, :Fc], True, True)
                    sQ = t3p.tile([128, 512], F32, tag="sig")
                    nc.scalar.activation(sQ[:, :Fc], Qp[:, :Fc], AF.Sigmoid,
                                         bias=b(f"bQ{i}"), scale=1.0)
                    u = t3p.tile([128, 512], F32R, tag="u")
                    nc.vector.scalar_tensor_tensor(
                        out=u[:, :Fc], in0=Pp[:, :Fc], scalar=b(f"bP{i}"),
                        in1=sQ[:, :Fc], op0=Alu.add, op1=Alu.mult)
                    Rp = ps.tile([128, 512], F32, tag="mm", bufs=CFG["mm"])
                    mm(Rp[:, :Fc], f"R{i}", q[:, :Fc], True, False)
                    mm(Rp[:, :Fc], "IBD", u[:, :Fc], False, True)
                    t = t3p.tile([128, 512], F32R, tag="t")
                    relu_ps(t[:, :Fc], Rp[:, :Fc], f"bR{i}", CFG["e_t"])

                    Cp = ps.tile([128, 512], F32, tag="mm", bufs=CFG["mm"])
                    mm(Cp[:, :Fc], f"CH{i}", t[:, :Fc], True, True)
                    tt = t3p.tile([128, 512], F32R, tag="t")
                    relu_ps(tt[:, :Fc], Cp[:, :Fc], f"bC{i}", CFG["e_t2"])

                    P2p = ps.tile([128, 512], F32, tag="mm", bufs=CFG["mm"])
                    mm(P2p[:, :Fc], f"P2{i}", tt[:, :Fc], True, True)
                    Q2p = ps.tile([128, 512], F32, tag="mm", bufs=CFG["mm"])
                    mm(Q2p[:, :Fc], f"Q2{i}", tt[:, :Fc], True, True)
                    sQ2 = t3p.tile([128, 512], F32, tag="sig")
                    nc.scalar.activation(sQ2[:, :Fc], Q2p[:, :Fc], AF.Sigmoid,
                                         bias=b(f"bQ2{i}"), scale=1.0)
                    u2 = t3p.tile([128, 512], F32R, tag="u")
                    nc.vector.scalar_tensor_tensor(
                        out=u2[:, :Fc], in0=P2p[:, :Fc], scalar=b(f"bP2{i}"),
                        in1=sQ2[:, :Fc], op0=Alu.add, op1=Alu.mult)
                    R2p = ps.tile([128, 512], F32, tag="mm", bufs=CFG["mm"])
                    mm(R2p[:, :Fc], f"R2{i}", tt[:, :Fc], True, False)
                    mm(R2p[:, :Fc], "IBD", u2[:, :Fc], False, True)
                    o = t5p.tile([128, 512], F32R, tag="o")
                    relu_ps(o[:, :Fc], R2p[:, :Fc], f"bR2{i}", CFG["e_o"])
                    st[c]["o"] = o

                    zp = ps.tile([128, 512], F32, tag="acc", bufs=CFG["acc"])
                    mm(zp[:, :Fc], "CBD", xn[:, :Fc], True, False)
                    for j in range(4):
                        fp = ps.tile([128, 512], F32, tag="mm", bufs=CFG["mm"])
                        mm(fp[:, :Fc], f"F1{i}_{j}", xn[:, :Fc], True, True)
                        fj = t3p.tile([128, 512], F32R, tag="fj")
                        relu_ps(fj[:, :Fc], fp[:, :Fc], f"b1{i}_{j}",
                                "act" if j < CFG["ff_act"] else "dve")
                        mm(zp[:, :Fc], f"F2{i}_{j}", fj[:, :Fc],
                           False, j == 3)
                    st[c]["z2s"] = ln_stats(zp, Fc, f"cb2{i}", qv2, k)

                yq2 = ln_rsqrt(qv2, top)

                # ---- mid: us, gating, blk stats ----
                qvb = qp.tile([128, 512], F32, tag="qv")
                for k, c in enumerate(group):
                    Fc = st[c]["Fc"]
                    q = st[c]["h"]
                    o = st[c]["o"]
                    us = t3p.tile([128, 512], F32R, tag="us")
                    ln_apply(st[c]["z2s"], yq2, k, Fc, us,
                             f"g2{i}", f"h2{i}", ln2t[i])
                    fgp = ps.tile([128, 512], F32, tag="mm", bufs=CFG["mm"])
                    mm(fgp[:, :Fc], f"FGQ{i}", q[:, :Fc], True, False)
                    mm(fgp[:, :Fc], f"FGO{i}", o[:, :Fc], False, True)
                    fgx = t3p.tile([128, 512], F32R, tag="fgx")
                    evac_ps(fgx[:, :Fc], fgp[:, :Fc], f"bFG{i}", CFG["e_fgx"])
                    fsp = ps.tile([128, 512], F32, tag="mm", bufs=CFG["mm"])
                    mm(fsp[:, :Fc], "IBD", fgx[:, :Fc], True, False)
                    mm(fsp[:, :Fc], f"FS{i}", us[:, :Fc], False, True)
                    g = t3p.tile([128, 512], F32, tag="g")
                    nc.scalar.activation(g[:, :Fc], fsp[:, :Fc], AF.Sigmoid,
                                         bias=b(f"bFS{i}"), scale=1.0)
                    dps = ps.tile([128, 512], F32, tag="mm", bufs=CFG["mm"])
                    mm(dps[:, :Fc], "IBD", us[:, :Fc], True, False)
                    mm(dps[:, :Fc], "IBDN", fgx[:, :Fc], False, True)
                    e = t3p.tile([128, 512], F32R, tag="e")
                    nc.vector.tensor_tensor(out=e[:, :Fc], in0=g[:, :Fc],
                                            in1=dps[:, :Fc], op=Alu.mult)
                    zb = ps.tile([128, 512], F32, tag="acc", bufs=CFG["acc"])
                    mm(zb[:, :Fc], "CBD", fgx[:, :Fc], True, False)
                    mm(zb[:, :Fc], "CBD", q[:, :Fc], False, False)
                    mm(zb[:, :Fc], "CBD", e[:, :Fc], False, True)
                    st[c]["zbs"] = ln_stats(zb, Fc, None, qvb, k)

                yqb = ln_rsqrt(qvb, top)

                # ---- tail: normalize into next h ----
                for k, c in enumerate(group):
                    Fc = st[c]["Fc"]
                    hn = t5p.tile([128, 512], F32R, tag="h", bufs=CFG["h"])
                    ln_apply(st[c]["zbs"], yqb, k, Fc, hn,
                             f"gB{i}", f"hB{i}", blkt[i])
                    st[c]["h"] = hn

            # ---- output projection ----
            for k, c in enumerate(group):
                Fc = st[c]["Fc"]
                yp = ps.tile([24, 512], F32, tag="acc", bufs=CFG["acc"])
                mm(yp[:, :Fc], "OUT", st[c]["h"][:, :Fc], True, True,
                   mslice=24)
                yt = io.tile([24, 512], F32, tag="yt")
                nc.scalar.activation(yt[:, :Fc], yp[:, :Fc], AF.Identity,
                                     bias=b("BOUT", 24), scale=1.0)
                nc.sync.dma_start(out=d_y.ap()[:, st[c]["sl"]],
                                  in_=yt[:, :Fc])

    _split_waits(nc, mybir)
    return nc


def _get_compiled(flags):
    if flags not in _COMPILED:
        _COMPILED[flags] = _build(flags)
    return _COMPILED[flags]


def kernel(x, edge_index, edge_weight, params):
    from concourse import bass_utils

    x = np.asarray(x, dtype=np.float32)
    p = {k: np.asarray(v, dtype=np.float32) for k, v in params.items()}

    flags = _make_flags(p)
    nc = _get_compiled(flags)

    W, B = _pack_host(p, flags)

    xpad = np.zeros((NPAD, F_IN), np.float32)
    xpad[:N] = x
    xc = xpad.reshape(NCORES, CN, F_IN)
    in_maps = []
    for c in range(NCORES):
        xd = np.ones((33, G), np.float32)
        xd[0:16] = xc[c, :G].T
        xd[16:32] = xc[c, G:].T
        in_maps.append({"x_packed": xd, "wstack": W, "bstack": B})

    res = bass_utils.run_bass_kernel_spmd(
        nc, in_maps, core_ids=list(range(NCORES)), trace=TRACE)
    global LAST_EXEC_NS, LAST_RESULTS
    LAST_EXEC_NS = res.exec_time_ns
    LAST_RESULTS = res

    out = np.zeros((NPAD, OUT_LEN), np.float32)
    xe = np.zeros((NPAD, D), np.float32)
    for c in range(NCORES):
        y = res.results[c]["y_out"]
        e = res.results[c]["xe_out"]
        base = c * CN
        out[base:base + G] = y[0:12].T
        out[base + G:base + CN] = y[12:24].T
        xe[base:base + G] = e[0:64].T
        xe[base + G:base + CN] = e[64:128].T
    return out[:N], xe[:N]


# revision 22
# speedup vs baseline: 1.0872x; 1.0415x over previous
"""Trainium2 Bass kernel for nn_Model_1778116460915 (gnn_message_passing).

Per-node MLP stack (ChebConv K=1 does no propagation; edge data unused):
  h = x @ We + be; x_embed = h; 3x {temporal convs, cheb, FF, layernorms,
  gated fusion}; out = h @ Wo + bo.  Returns (out, x_embed).

Strategy:
  - Nodes sharded across 8 cores (12544/core, N padded to 100352).
  - Feature-major on-chip layout: [128 partitions, nodes] tiles = 2 node
    groups x 64 features; all matmuls are 128x128 block-diagonal float32r
    (FP22) at full PE rate.
  - LayerNorm: mean-centering folded into weights (C = I - J/64), variance
    via squares + ones-matmul, rsqrt via integer magic-constant seed
    (int32<->fp32 ACT port conversion) + 2 Newton steps; the rsqrt chain is
    shared across groups of 4 chunks (stats DMA-gathered into partition
    rows 0/32/64/96 of one tile) to amortize per-op cost.
  - eps folding: stats weights pre-scaled by 1/64; the reference's +eps in
    rsqrt(var+eps) is dropped (relative effect <= ~1e-4 for this model's
    variance ranges, far below the fp22 matmul noise).
  - Elementwise work balanced across ACT / DVE / GPSIMD engines.
"""
import sys
import numpy as np

for _p in ('/opt/trn_rl_repo', '/root/.axon_site/_ro/trn_rl_repo'):
    if _p not in sys.path:
        sys.path.append(_p)

N = 100000
F_IN = 16
D = 64
FF = 256
OUT_LEN = 12
L = 3
EPS = 1e-5

NCORES = 8
CN = 12544              # nodes per core (padded)
NPAD = NCORES * CN      # 100352
G = CN // 2             # free-dim length per core (2 groups packed) = 6272
CHUNKS = [512] * 11 + [320, 320]       # sum = 6272; each >=256 (f32r rate)
GROUPS = [[0, 1, 2, 3], [4, 5, 6, 7], [8, 9, 10, 11], [12]]
MAGIC = 1597463007.0    # 0x5F3759DF
NR_ITERS = 2
CFG = {"mm": 4, "acc": 2, "stat": 1, "bc": 1, "t3": 4, "t5": 5, "h": 10,
       "e_t": "act", "e_t2": "act", "e_o": "act", "ff_act": 2,
       "e_fgx": "act", "e_z0s": "act", "e_stat": "dve", "e_sq": "act"}

_COMPILED = {}
TRACE = False
LAST_EXEC_NS = None
LAST_RESULTS = None


def _slot_layout(flags):
    wslots = ["CBD", "SBD", "BBD", "IBD", "IBDN", "EMB", "OUT"]
    bslots = ["MAGIC", "BOUT"]
    for i in range(L):
        wslots += [f"P{i}", f"Q{i}", f"R{i}", f"CH{i}",
                   f"P2{i}", f"Q2{i}", f"R2{i}"]
        wslots += [f"F1{i}_{j}" for j in range(4)]
        wslots += [f"F2{i}_{j}" for j in range(4)]
        wslots += [f"FS{i}", f"FGQ{i}", f"FGO{i}"]
        bslots += [f"bP{i}", f"bQ{i}", f"bR{i}", f"bC{i}",
                   f"bP2{i}", f"bQ2{i}", f"bR2{i}"]
        bslots += [f"b1{i}_{j}" for j in range(4)]
        bslots += [f"cb2{i}", f"bFG{i}", f"bFS{i}",
                   f"g1{i}", f"h1{i}", f"g2{i}", f"h2{i}",
                   f"gB{i}", f"hB{i}"]
    wi = {n: k for k, n in enumerate(wslots)}
    bi = {n: k for k, n in enumerate(bslots)}
    return wslots, bslots, wi, bi


def _bd(a):
    z = np.zeros((128, 128), np.float32)
    z[0:64, 0:64] = a
    z[64:128, 64:128] = a
    return z


def _dup(v):
    return np.concatenate([v, v]).astype(np.float32)


def _trivial(g, b):
    return bool(np.all(g == 1.0) and np.all(b == 0.0))


def _make_flags(p):
    ln1t = [_trivial(p["ln1_g"][i], p["ln1_b"][i]) for i in range(L)]
    ln2t = [_trivial(p["ln2_g"][i], p["ln2_b"][i]) for i in range(L)]
    blkt = [_trivial(p["blk_g"][i], p["blk_b"][i]) for i in range(L)]
    ln1_full = tuple(
        (i == 0) or (not ln1t[i]) or (not blkt[i - 1]) for i in range(L))
    return ln1_full, tuple(ln2t), tuple(blkt), tuple(ln1t)


def _pack_host(p, flags):
    wslots, bslots, wi, bi = _slot_layout(flags)
    C = np.eye(64, dtype=np.float32) - np.float32(1.0 / 64.0)
    W = np.zeros((128, len(wslots) * 128), np.float32)
    B = np.zeros((128, len(bslots)), np.float32)

    def setw(name, m):
        k = wi[name]
        W[:, k * 128:(k + 1) * 128] = m

    def setb(name, v):
        B[:, bi[name]] = v

    setw("CBD", _bd(C))
    sbd = np.zeros((128, 128), np.float32)
    sbd[0:64, 0] = 1.0 / 64.0          # stats matmul yields variance directly
    sbd[64:128, 1] = 1.0 / 64.0
    setw("SBD", sbd)
    bbd = np.zeros((128, 128), np.float32)
    for k in range(4):                  # replicated for quad rows 0/32/64/96
        bbd[32 * k, 0:64] = 1.0
        bbd[32 * k + 1, 64:128] = 1.0
    setw("BBD", bbd)
    setw("IBD", np.eye(128, dtype=np.float32))
    setw("IBDN", -np.eye(128, dtype=np.float32))
    emb = np.zeros((128, 128), np.float32)
    emb[0:16, 0:64] = p["embed_w"]
    emb[16:32, 64:128] = p["embed_w"]
    emb[32, 0:64] = p["embed_b"]
    emb[32, 64:128] = p["embed_b"]
    setw("EMB", emb)
    ow = np.zeros((128, 128), np.float32)
    ow[0:64, 0:12] = p["out_w"]
    ow[64:128, 12:24] = p["out_w"]
    setw("OUT", ow)

    B[:, bi["MAGIC"]] = MAGIC
    bo = np.zeros(128, np.float32)
    bo[0:12] = p["out_b"]
    bo[12:24] = p["out_b"]
    setb("BOUT", bo)

    for i in range(L):
        setw(f"P{i}", _bd(p["tc1_w"][i, 0]))
        setw(f"Q{i}", _bd(p["tc1_w"][i, 1]))
        setw(f"R{i}", _bd(p["tc1_w"][i, 2]))
        setw(f"CH{i}", _bd(p["cheb_w"][i]))
        setw(f"P2{i}", _bd(p["tc2_w"][i, 0]))
        setw(f"Q2{i}", _bd(p["tc2_w"][i, 1]))
        setw(f"R2{i}", _bd(p["tc2_w"][i, 2]))
        for j in range(4):
            setw(f"F1{i}_{j}", _bd(p["ff_w1"][i][:, 64 * j:64 * (j + 1)]))
        W2C = (p["ff_w2"][i] @ C).astype(np.float32)
        for j in range(4):
            setw(f"F2{i}_{j}", _bd(W2C[64 * j:64 * (j + 1), :]))
        setw(f"FS{i}", _bd(p["fs_w"][i]))
        setw(f"FGQ{i}", _bd(p["fg_w"][i][0:64, :]))
        setw(f"FGO{i}", _bd(p["fg_w"][i][64:128, :]))

        setb(f"bP{i}", _dup(p["tc1_b"][i, 0]))
        setb(f"bQ{i}", _dup(p["tc1_b"][i, 1]))
        setb(f"bR{i}", _dup(p["tc1_b"][i, 2]))
        setb(f"bC{i}", _dup(p["cheb_b"][i]))
        setb(f"bP2{i}", _dup(p["tc2_b"][i, 0]))
        setb(f"bQ2{i}", _dup(p["tc2_b"][i, 1]))
        setb(f"bR2{i}", _dup(p["tc2_b"][i, 2]))
        for j in range(4):
            setb(f"b1{i}_{j}", _dup(p["ff_b1"][i][64 * j:64 * (j + 1)]))
        setb(f"cb2{i}", _dup(C @ p["ff_b2"][i]))
        setb(f"bFG{i}", _dup(p["fg_b"][i]))
        setb(f"bFS{i}", _dup(p["fs_b"][i]))
        setb(f"g1{i}", _dup(p["ln1_g"][i]))
        setb(f"h1{i}", _dup(p["ln1_b"][i]))
        setb(f"g2{i}", _dup(p["ln2_g"][i]))
        setb(f"h2{i}", _dup(p["ln2_b"][i]))
        setb(f"gB{i}", _dup(p["blk_g"][i]))
        setb(f"hB{i}", _dup(p["blk_b"][i]))
    return W, B


def _split_waits(nc, mybir):
    """walrus here encodes at most ONE semaphore wait per instruction; move
    extras onto engine-matched NoOps inserted right before the instruction."""
    n = 0
    for f in nc.m.functions:
        for blk in f.blocks:
            out = []
            changed = False
            for inst in blk.instructions:
                si = inst.sync_info
                waits = list(si.on_wait) if si is not None and si.on_wait else []
                if len(waits) > 1:
                    for w in waits[:-1]:
                        nop = mybir.InstNoOp(
                            name=f"{inst.name}-wsplit{n}", ins=[], outs=[])
                        nop.engine = inst.engine
                        nop.sync_info = mybir.SyncInfo(on_wait=[w], on_update=[])
                        out.append(nop)
                        n += 1
                    si.on_wait = [waits[-1]]
                    inst.sync_info = si
                    changed = True
                out.append(inst)
            if changed:
                blk.instructions = out
    return n


def _build(flags):
    import contextlib
    import concourse.bass as bass
    import concourse.mybir as mybir
    import concourse.tile as tile

    ln1_full, ln2t, blkt, ln1t = flags
    wslots, bslots, wi, bi = _slot_layout(flags)
    NW = len(wslots)
    NB = len(bslots)

    F32 = mybir.dt.float32
    F32R = mybir.dt.float32r
    I32 = mybir.dt.int32
    AF = mybir.ActivationFunctionType
    Alu = mybir.AluOpType

    nc = bass.Bass("TRN2", target_bir_lowering=False, debug=False,
                   num_devices=NCORES)
    d_x = nc.dram_tensor("x_packed", [33, G], F32R, kind="ExternalInput")
    d_w = nc.dram_tensor("wstack", [128, NW * 128], F32R, kind="ExternalInput")
    d_b = nc.dram_tensor("bstack", [128, NB], F32, kind="ExternalInput")
    d_y = nc.dram_tensor("y_out", [24, G], F32, kind="ExternalOutput")
    d_xe = nc.dram_tensor("xe_out", [128, G], F32, kind="ExternalOutput")

    with tile.TileContext(nc) as tc, contextlib.ExitStack() as ctx:
        wpool = ctx.enter_context(tc.tile_pool(name="wpool", bufs=1))
        io = ctx.enter_context(tc.tile_pool(name="io", bufs=3))
        t2p = ctx.enter_context(tc.tile_pool(name="t2p", bufs=2))
        t3p = ctx.enter_context(tc.tile_pool(name="t3p", bufs=CFG["t3"]))
        t5p = ctx.enter_context(tc.tile_pool(name="t5p", bufs=CFG["t5"]))
        qp = ctx.enter_context(tc.tile_pool(name="qp", bufs=2))
        ps = ctx.enter_context(tc.tile_pool(name="ps", bufs=2, space="PSUM"))

        WS = wpool.tile([128, NW * 128], F32R)
        BS = wpool.tile([128, NB], F32)
        nc.sync.dma_start(out=BS, in_=d_b.ap())
        # globals + layer-0 weights first so the embed/layer-0 pipeline can
        # start while the rest of the weight stack streams in
        w_split = 24 * 128
        nc.sync.dma_start(out=WS[:, :w_split], in_=d_w.ap()[:, :w_split])
        nc.sync.dma_start(out=WS[:, w_split:], in_=d_w.ap()[:, w_split:])

        def w(name, kslice=None, mslice=None):
            k = wi[name]
            a = WS[:, k * 128:(k + 1) * 128]
            if kslice is not None:
                a = a[kslice[0]:kslice[1]]
            if mslice is not None:
                a = a[:, 0:mslice]
            return a

        def b(name, parts=128):
            return BS[0:parts, bi[name]:bi[name] + 1]

        def mm(out, name, rhs, start, stop, kslice=None, mslice=None,
               tile_position=None):
            nc.tensor.matmul(out, w(name, kslice, mslice), rhs,
                             start=start, stop=stop,
                             tile_position=tile_position)

        def relu_ps(out_ap, ps_ap, biascol, eng):
            if eng == "act":
                nc.scalar.activation(out_ap, ps_ap, AF.Relu,
                                     bias=b(biascol), scale=1.0)
            else:
                nc.vector.tensor_scalar(
                    out=out_ap, in0=ps_ap, scalar1=b(biascol), scalar2=0.0,
                    op0=Alu.add, op1=Alu.max)

        def evac_ps(out_ap, ps_ap, biascol, eng):
            if eng == "act":
                if biascol is None:
                    nc.scalar.copy(out_ap, ps_ap)
                else:
                    nc.scalar.activation(out_ap, ps_ap, AF.Identity,
                                         bias=b(biascol), scale=1.0)
            else:
                if biascol is None:
                    nc.vector.tensor_copy(out_ap, ps_ap)
                else:
                    nc.vector.tensor_scalar(
                        out=out_ap, in0=ps_ap, scalar1=b(biascol),
                        scalar2=None, op0=Alu.add)

        FCS = [CHUNKS[c] for c in range(len(CHUNKS))]

        def ln_stats(zp, Fc, cbcol, quadv, k):
            """Evacuate centered zp, square, reduce -> variance row-pair k of
            quadv. Returns z0s (SBUF, centered values incl. bias)."""
            z0s = t5p.tile([128, 512], F32, tag="z0s")
            evac_ps(z0s[:, :Fc], zp[:, :Fc], cbcol, CFG["e_z0s"])
            sq = t3p.tile([128, 512], F32R, tag="sq")
            if CFG["e_sq"] == "act":
                nc.scalar.activation(
                    sq[:, :Fc], zp[:, :Fc], AF.Square,
                    bias=(b(cbcol) if cbcol is not None else 0.0), scale=1.0)
            else:
                nc.gpsimd.tensor_tensor(out=sq[:, :Fc], in0=z0s[:, :Fc],
                                        in1=z0s[:, :Fc], op=Alu.mult)
            st = ps.tile([2, 512], F32, tag="stat", bufs=CFG["stat"])
            nc.tensor.matmul(st[:, :Fc], w("SBD", mslice=2), sq[:, :Fc],
                             start=True, stop=True)
            if k == 0:
                evac_ps(quadv[0:2, :Fc], st[:, :Fc], None, CFG["e_stat"])
            else:
                vt = t3p.tile([2, 512], F32, tag="vt")
                evac_ps(vt[:, :Fc], st[:, :Fc], None, CFG["e_stat"])
                nc.sync.dma_start(out=quadv[32 * k:32 * k + 2, :Fc],
                                  in_=vt[:, :Fc])
            return z0s

        def ln_rsqrt(quadv, top):
            """Shared seed + Newton over quad rows; returns y tile (F32R)."""
            seed = qp.tile([128, 512], I32, tag="qs")
            nc.scalar.activation(seed[0:top], quadv[0:top].bitcast(I32),
                                 AF.Identity, bias=b("MAGIC", top), scale=-0.5)
            y = seed.bitcast(F32)
            for it in range(NR_ITERS):
                last = it == NR_ITERS - 1
                a1 = qp.tile([128, 512], F32, tag="qa")
                nc.vector.tensor_tensor(out=a1[0:top], in0=y[0:top],
                                        in1=y[0:top], op=Alu.mult)
                b1 = qp.tile([128, 512], F32, tag="qb")
                nc.vector.scalar_tensor_tensor(
                    out=b1[0:top], in0=a1[0:top], scalar=-0.5,
                    in1=quadv[0:top], op0=Alu.mult, op1=Alu.mult)
                y2 = qp.tile([128, 512], F32R if last else F32, tag="qy")
                nc.vector.scalar_tensor_tensor(
                    out=y2[0:top], in0=b1[0:top], scalar=1.5,
                    in1=y[0:top], op0=Alu.add, op1=Alu.mult)
                y = y2
            return y

        def ln_apply(z0s, yq, k, Fc, out_tile, gcol, hcol, trivial_affine):
            rb = ps.tile([128, 512], F32, tag="bc", bufs=CFG["bc"])
            tp = (96, 0) if k == 3 else None
            nc.tensor.matmul(rb[:, :Fc],
                             w("BBD")[32 * k:32 * k + 2],
                             yq[32 * k:32 * k + 2, :Fc],
                             start=True, stop=True, tile_position=tp)
            if trivial_affine:
                nc.vector.tensor_tensor(out=out_tile[:, :Fc],
                                        in0=z0s[:, :Fc], in1=rb[:, :Fc],
                                        op=Alu.mult)
            else:
                pre = t3p.tile([128, 512], F32, tag="lnpre")
                nc.vector.tensor_tensor(out=pre[:, :Fc], in0=z0s[:, :Fc],
                                        in1=rb[:, :Fc], op=Alu.mult)
                nc.vector.tensor_scalar(
                    out=out_tile[:, :Fc], in0=pre[:, :Fc],
                    scalar1=b(gcol), scalar2=b(hcol),
                    op0=Alu.mult, op1=Alu.add)

        # ================= main loop: groups of chunks =================
        for group in GROUPS:
            top = 32 * (len(group) - 1) + 2
            st = {}
            # ---- embed ----
            for k, c in enumerate(group):
                Fc = FCS[c]
                sl = slice(sum(FCS[:c]), sum(FCS[:c]) + Fc)
                xt = io.tile([33, 512], F32R, tag="xt")
                nc.sync.dma_start(out=xt[:, :Fc], in_=d_x.ap()[:, sl])
                ep = ps.tile([128, 512], F32, tag="acc", bufs=CFG["acc"])
                mm(ep[:, :Fc], "EMB", xt[:, :Fc], True, True, kslice=(0, 33))
                h = t5p.tile([128, 512], F32R, tag="h", bufs=CFG["h"])
                nc.scalar.copy(h[:, :Fc], ep[:, :Fc])
                nc.sync.dma_start(out=d_xe.ap()[:, sl],
                                  in_=h[:, :Fc].bitcast(F32))
                st[c] = {"h": h, "sl": sl, "Fc": Fc}

            for i in range(L):
                # ---- LN1 (full only when needed) ----
                if ln1_full[i]:
                    qv1 = qp.tile([128, 512], F32, tag="qv")
                    for k, c in enumerate(group):
                        Fc = st[c]["Fc"]
                        z1 = ps.tile([128, 512], F32, tag="acc", bufs=CFG["acc"])
                        mm(z1[:, :Fc], "CBD", st[c]["h"][:, :Fc], True, True)
                        st[c]["z1s"] = ln_stats(z1, Fc, None, qv1, k)
                    yq1 = ln_rsqrt(qv1, top)
                    for k, c in enumerate(group):
                        Fc = st[c]["Fc"]
                        xn = t5p.tile([128, 512], F32R, tag="xn")
                        ln_apply(st[c]["z1s"], yq1, k, Fc, xn,
                                 f"g1{i}", f"h1{i}", ln1t[i])
                        st[c]["xn"] = xn
                else:
                    for c in group:
                        st[c]["xn"] = st[c]["h"]

                # ---- front: tc1, cheb, tc2, ff -> LN2 stats ----
                qv2 = qp.tile([128, 512], F32, tag="qv")
                for k, c in enumerate(group):
                    Fc = st[c]["Fc"]
                    q = st[c]["h"]
                    xn = st[c]["xn"]
                    Pp = ps.tile([128, 512], F32, tag="mm", bufs=CFG["mm"])
                    mm(Pp[:, :Fc], f"P{i}", q[:, :Fc], True, True)
                    Qp = ps.tile([128, 512], F32, tag="mm", bufs=CFG["mm"])
                    mm(Qp[:, :Fc], f"Q{i}", q[:, :Fc], True, True)
                    sQ = t3p.tile([128, 512], F32, tag="sig")
                    nc.scalar.activation(sQ[:, :Fc], Qp[:, :Fc], AF.Sigmoid,
                                         bias=b(f"bQ{i}"), scale=1.0)
                    u = t3p.tile([128, 512], F32R, tag="u")
                    nc.vector.scalar_tensor_tensor(
                        out=u[:, :Fc], in0=Pp[:, :Fc], scalar=b(f"bP{i}"),
                        in1=sQ[:, :Fc], op0=Alu.add, op1=Alu.mult)
                    Rp = ps.tile([128, 512], F32, tag="mm", bufs=CFG["mm"])
                    mm(Rp[:, :Fc], f"R{i}", q[:, :Fc], True, False)
                    mm(Rp[:, :Fc], "IBD", u[:, :Fc], False, True)
                    t = t3p.tile([128, 512], F32R, tag="t")
                    relu_ps(t[:, :Fc], Rp[:, :Fc], f"bR{i}", CFG["e_t"])

                    Cp = ps.tile([128, 512], F32, tag="mm", bufs=CFG["mm"])
                    mm(Cp[:, :Fc], f"CH{i}", t[:, :Fc], True, True)
                    tt = t3p.tile([128, 512], F32R, tag="t")
                    relu_ps(tt[:, :Fc], Cp[:, :Fc], f"bC{i}", CFG["e_t2"])

                    P2p = ps.tile([128, 512], F32, tag="mm", bufs=CFG["mm"])
                    mm(P2p[:, :Fc], f"P2{i}", tt[:, :Fc], True, True)
                    Q2p = ps.tile([128, 512], F32, tag="mm", bufs=CFG["mm"])
                    mm(Q2p[:, :Fc], f"Q2{i}", tt[:, :Fc], True, True)
                    sQ2 = t3p.tile([128, 512], F32, tag="sig")
                    nc.scalar.activation(sQ2[:, :Fc], Q2p[:, :Fc], AF.Sigmoid,
                                         bias=b(f"bQ2{i}"), scale=1.0)
                    u2 = t3p.tile([128, 512], F32R, tag="u")
                    nc.vector.scalar_tensor_tensor(
                        out=u2[:, :Fc], in0=P2p[:, :Fc], scalar=b(f"bP2{i}"),
                        in1=sQ2[:, :Fc], op0=Alu.add, op1=Alu.mult)
                    R2p = ps.tile([128, 512], F32, tag="mm", bufs=CFG["mm"])
                    mm(R2p[:, :Fc], f"R2{i}", tt[:, :Fc], True, False)
                    mm(R2p[:, :Fc], "IBD", u2[:, :Fc], False, True)
                    o = t5p.tile([128, 512], F32R, tag="o")
                    relu_ps(o[:, :Fc], R2p[:, :Fc], f"bR2{i}", CFG["e_o"])
                    st[c]["o"] = o

                    zp = ps.tile([128, 512], F32, tag="acc", bufs=CFG["acc"])
                    mm(zp[:, :Fc], "CBD", xn[:, :Fc], True, False)
                    for j in range(4):
                        fp = ps.tile([128, 512], F32, tag="mm", bufs=CFG["mm"])
                        mm(fp[:, :Fc], f"F1{i}_{j}", xn[:, :Fc], True, True)
                        fj = t3p.tile([128, 512], F32R, tag="fj")
                        relu_ps(fj[:, :Fc], fp[:, :Fc], f"b1{i}_{j}",
                                "act" if j < CFG["ff_act"] else "dve")
                        mm(zp[:, :Fc], f"F2{i}_{j}", fj[:, :Fc],
                           False, j == 3)
                    st[c]["z2s"] = ln_stats(zp, Fc, f"cb2{i}", qv2, k)

                yq2 = ln_rsqrt(qv2, top)

                # ---- mid: us, gating, blk stats ----
                qvb = qp.tile([128, 512], F32, tag="qv")
                for k, c in enumerate(group):
                    Fc = st[c]["Fc"]
                    q = st[c]["h"]
                    o = st[c]["o"]
                    us = t3p.tile([128, 512], F32R, tag="us")
                    ln_apply(st[c]["z2s"], yq2, k, Fc, us,
                             f"g2{i}", f"h2{i}", ln2t[i])
                    fgp = ps.tile([128, 512], F32, tag="mm", bufs=CFG["mm"])
                    mm(fgp[:, :Fc], f"FGQ{i}", q[:, :Fc], True, False)
                    mm(fgp[:, :Fc], f"FGO{i}", o[:, :Fc], False, True)
                    fgx = t3p.tile([128, 512], F32R, tag="fgx")
                    evac_ps(fgx[:, :Fc], fgp[:, :Fc], f"bFG{i}", CFG["e_fgx"])
                    fsp = ps.tile([128, 512], F32, tag="mm", bufs=CFG["mm"])
                    mm(fsp[:, :Fc], "IBD", fgx[:, :Fc], True, False)
                    mm(fsp[:, :Fc], f"FS{i}", us[:, :Fc], False, True)
                    g = t3p.tile([128, 512], F32, tag="g")
                    nc.scalar.activation(g[:, :Fc], fsp[:, :Fc], AF.Sigmoid,
                                         bias=b(f"bFS{i}"), scale=1.0)
                    dd = t3p.tile([128, 512], F32, tag="e")
                    nc.gpsimd.tensor_tensor(out=dd[:, :Fc],
                                            in0=us[:, :Fc].bitcast(F32),
                                            in1=fgx[:, :Fc].bitcast(F32),
                                            op=Alu.subtract)
                    e = t3p.tile([128, 512], F32R, tag="u")
                    nc.vector.tensor_tensor(out=e[:, :Fc], in0=g[:, :Fc],
                                            in1=dd[:, :Fc], op=Alu.mult)
                    zb = ps.tile([128, 512], F32, tag="acc", bufs=CFG["acc"])
                    mm(zb[:, :Fc], "CBD", fgx[:, :Fc], True, False)
                    mm(zb[:, :Fc], "CBD", q[:, :Fc], False, False)
                    mm(zb[:, :Fc], "CBD", e[:, :Fc], False, True)
                    st[c]["zbs"] = ln_stats(zb, Fc, None, qvb, k)

                yqb = ln_rsqrt(qvb, top)

                # ---- tail: normalize into next h ----
                for k, c in enumerate(group):
                    Fc = st[c]["Fc"]
                    hn = t5p.tile([128, 512], F32R, tag="h", bufs=CFG["h"])
                    ln_apply(st[c]["zbs"], yqb, k, Fc, hn,
                             f"gB{i}", f"hB{i}", blkt[i])
                    st[c]["h"] = hn

            # ---- output projection ----
            for k, c in enumerate(group):
                Fc = st[c]["Fc"]
                yp = ps.tile([24, 512], F32, tag="acc", bufs=CFG["acc"])
                mm(yp[:, :Fc], "OUT", st[c]["h"][:, :Fc], True, True,
                   mslice=24)
                yt = io.tile([24, 512], F32, tag="yt")
                nc.scalar.activation(yt[:, :Fc], yp[:, :Fc], AF.Identity,
                                     bias=b("BOUT", 24), scale=1.0)
                nc.sync.dma_start(out=d_y.ap()[:, st[c]["sl"]],
                                  in_=yt[:, :Fc])

    _split_waits(nc, mybir)
    return nc


def _get_compiled(flags):
    if flags not in _COMPILED:
        _COMPILED[flags] = _build(flags)
    return _COMPILED[flags]


def kernel(x, edge_index, edge_weight, params):
    from concourse import bass_utils

    x = np.asarray(x, dtype=np.float32)
    p = {k: np.asarray(v, dtype=np.float32) for k, v in params.items()}

    flags = _make_flags(p)
    nc = _get_compiled(flags)

    W, B = _pack_host(p, flags)

    xpad = np.zeros((NPAD, F_IN), np.float32)
    xpad[:N] = x
    xc = xpad.reshape(NCORES, CN, F_IN)
    in_maps = []
    for c in range(NCORES):
        xd = np.ones((33, G), np.float32)
        xd[0:16] = xc[c, :G].T
        xd[16:32] = xc[c, G:].T
        in_maps.append({"x_packed": xd, "wstack": W, "bstack": B})

    res = bass_utils.run_bass_kernel_spmd(
        nc, in_maps, core_ids=list(range(NCORES)), trace=TRACE)
    global LAST_EXEC_NS, LAST_RESULTS
    LAST_EXEC_NS = res.exec_time_ns
    LAST_RESULTS = res

    out = np.zeros((NPAD, OUT_LEN), np.float32)
    xe = np.zeros((NPAD, D), np.float32)
    for c in range(NCORES):
        y = res.results[c]["y_out"]
        e = res.results[c]["xe_out"]
        base = c * CN
        out[base:base + G] = y[0:12].T
        out[base + G:base + CN] = y[12:24].T
        xe[base:base + G] = e[0:64].T
        xe[base + G:base + CN] = e[64:128].T
    return out[:N], xe[:N]


# revision 24
# speedup vs baseline: 1.1747x; 1.0805x over previous
"""Trainium2 Bass kernel for nn_Model_1778116460915 (gnn_message_passing).

Per-node MLP stack (ChebConv K=1 does no propagation; edge data unused):
  h = x @ We + be; x_embed = h; 3x {temporal convs, cheb, FF, layernorms,
  gated fusion}; out = h @ Wo + bo.  Returns (out, x_embed).

Strategy:
  - Nodes sharded across 8 cores (12544/core, N padded to 100352).
  - Feature-major on-chip layout: [128 partitions, nodes] tiles = 2 node
    groups x 64 features; all matmuls are 128x128 block-diagonal float32r
    (FP22) at full PE rate.
  - LayerNorm: mean-centering folded into weights (C = I - J/64), variance
    via squares + ones-matmul, rsqrt via integer magic-constant seed
    (int32<->fp32 ACT port conversion) + 2 Newton steps; the rsqrt chain is
    shared across groups of 4 chunks (stats DMA-gathered into partition
    rows 0/32/64/96 of one tile) to amortize per-op cost.
  - eps folding: stats weights pre-scaled by 1/64; the reference's +eps in
    rsqrt(var+eps) is dropped (relative effect <= ~1e-4 for this model's
    variance ranges, far below the fp22 matmul noise).
  - Elementwise work balanced across ACT / DVE / GPSIMD engines.
"""
import sys
import numpy as np

for _p in ('/opt/trn_rl_repo', '/root/.axon_site/_ro/trn_rl_repo'):
    if _p not in sys.path:
        sys.path.append(_p)

N = 100000
F_IN = 16
D = 64
FF = 256
OUT_LEN = 12
L = 3
EPS = 1e-5

NCORES = 8
CN = 12544              # nodes per core (padded)
NPAD = NCORES * CN      # 100352
G = CN // 2             # free-dim length per core (2 groups packed) = 6272
CHUNKS = [512] * 11 + [320, 320]       # sum = 6272; each >=256 (f32r rate)
GROUPS = [[0, 1, 2, 3], [4, 5, 6, 7], [8, 9, 10, 11], [12]]
MAGIC = 1597463007.0    # 0x5F3759DF
NR_ITERS = 2
CFG = {"mm": 4, "acc": 2, "stat": 1, "bc": 1, "t3": 4, "t5": 5, "h": 10,
       "e_t": "act", "e_t2": "act", "e_o": "act", "ff_act": 2,
       "e_fgx": "act", "e_z0s": "act", "e_stat": "dve", "e_sq": "act"}

_COMPILED = {}
TRACE = False
LAST_EXEC_NS = None
LAST_RESULTS = None


def _slot_layout(flags):
    wslots = ["CBD", "SBD", "BBD", "IBD", "IBDN", "EMB", "OUT"]
    bslots = ["MAGIC", "BOUT"]
    for i in range(L):
        wslots += [f"P{i}", f"Q{i}", f"R{i}", f"CH{i}",
                   f"P2{i}", f"Q2{i}", f"R2{i}"]
        wslots += [f"F1{i}_{j}" for j in range(4)]
        wslots += [f"F2{i}_{j}" for j in range(4)]
        wslots += [f"FS{i}", f"FGQ{i}", f"FGO{i}"]
        bslots += [f"bP{i}", f"bQ{i}", f"bR{i}", f"bC{i}",
                   f"bP2{i}", f"bQ2{i}", f"bR2{i}"]
        bslots += [f"b1{i}_{j}" for j in range(4)]
        bslots += [f"cb2{i}", f"bFG{i}", f"bFS{i}",
                   f"g1{i}", f"h1{i}", f"g2{i}", f"h2{i}",
                   f"gB{i}", f"hB{i}"]
    wi = {n: k for k, n in enumerate(wslots)}
    bi = {n: k for k, n in enumerate(bslots)}
    return wslots, bslots, wi, bi


def _bd(a):
    z = np.zeros((128, 128), np.float32)
    z[0:64, 0:64] = a
    z[64:128, 64:128] = a
    return z


def _dup(v):
    return np.concatenate([v, v]).astype(np.float32)


def _trivial(g, b):
    return bool(np.all(g == 1.0) and np.all(b == 0.0))


def _make_flags(p):
    ln1t = [_trivial(p["ln1_g"][i], p["ln1_b"][i]) for i in range(L)]
    ln2t = [_trivial(p["ln2_g"][i], p["ln2_b"][i]) for i in range(L)]
    blkt = [_trivial(p["blk_g"][i], p["blk_b"][i]) for i in range(L)]
    ln1_full = tuple(
        (i == 0) or (not ln1t[i]) or (not blkt[i - 1]) for i in range(L))
    return ln1_full, tuple(ln2t), tuple(blkt), tuple(ln1t)


def _pack_host(p, flags):
    wslots, bslots, wi, bi = _slot_layout(flags)
    C = np.eye(64, dtype=np.float32) - np.float32(1.0 / 64.0)
    W = np.zeros((128, len(wslots) * 128), np.float32)
    B = np.zeros((128, len(bslots)), np.float32)

    def setw(name, m):
        k = wi[name]
        W[:, k * 128:(k + 1) * 128] = m

    def setb(name, v):
        B[:, bi[name]] = v

    setw("CBD", _bd(C))
    sbd = np.zeros((128, 128), np.float32)
    sbd[0:64, 0] = 1.0 / 64.0          # stats matmul yields variance directly
    sbd[64:128, 1] = 1.0 / 64.0
    setw("SBD", sbd)
    bbd = np.zeros((128, 128), np.float32)
    for k in range(4):                  # replicated for quad rows 0/32/64/96
        bbd[32 * k, 0:64] = 1.0
        bbd[32 * k + 1, 64:128] = 1.0
    setw("BBD", bbd)
    setw("IBD", np.eye(128, dtype=np.float32))
    setw("IBDN", -np.eye(128, dtype=np.float32))
    emb = np.zeros((128, 128), np.float32)
    emb[0:16, 0:64] = p["embed_w"]
    emb[16:32, 64:128] = p["embed_w"]
    emb[32, 0:64] = p["embed_b"]
    emb[32, 64:128] = p["embed_b"]
    setw("EMB", emb)
    ow = np.zeros((128, 128), np.float32)
    ow[0:64, 0:12] = p["out_w"]
    ow[64:128, 12:24] = p["out_w"]
    setw("OUT", ow)

    B[:, bi["MAGIC"]] = MAGIC
    bo = np.zeros(128, np.float32)
    bo[0:12] = p["out_b"]
    bo[12:24] = p["out_b"]
    setb("BOUT", bo)

    for i in range(L):
        setw(f"P{i}", _bd(p["tc1_w"][i, 0]))
        setw(f"Q{i}", _bd(p["tc1_w"][i, 1]))
        setw(f"R{i}", _bd(p["tc1_w"][i, 2]))
        setw(f"CH{i}", _bd(p["cheb_w"][i]))
        setw(f"P2{i}", _bd(p["tc2_w"][i, 0]))
        setw(f"Q2{i}", _bd(p["tc2_w"][i, 1]))
        setw(f"R2{i}", _bd(p["tc2_w"][i, 2]))
        for j in range(4):
            setw(f"F1{i}_{j}", _bd(p["ff_w1"][i][:, 64 * j:64 * (j + 1)]))
        W2C = (p["ff_w2"][i] @ C).astype(np.float32)
        for j in range(4):
            setw(f"F2{i}_{j}", _bd(W2C[64 * j:64 * (j + 1), :]))
        setw(f"FS{i}", _bd(p["fs_w"][i]))
        setw(f"FGQ{i}", _bd(p["fg_w"][i][0:64, :]))
        setw(f"FGO{i}", _bd(p["fg_w"][i][64:128, :]))

        setb(f"bP{i}", _dup(p["tc1_b"][i, 0]))
        setb(f"bQ{i}", _dup(p["tc1_b"][i, 1]))
        setb(f"bR{i}", _dup(p["tc1_b"][i, 2]))
        setb(f"bC{i}", _dup(p["cheb_b"][i]))
        setb(f"bP2{i}", _dup(p["tc2_b"][i, 0]))
        setb(f"bQ2{i}", _dup(p["tc2_b"][i, 1]))
        setb(f"bR2{i}", _dup(p["tc2_b"][i, 2]))
        for j in range(4):
            setb(f"b1{i}_{j}", _dup(p["ff_b1"][i][64 * j:64 * (j + 1)]))
        setb(f"cb2{i}", _dup(C @ p["ff_b2"][i]))
        setb(f"bFG{i}", _dup(p["fg_b"][i]))
        setb(f"bFS{i}", _dup(p["fs_b"][i]))
        setb(f"g1{i}", _dup(p["ln1_g"][i]))
        setb(f"h1{i}", _dup(p["ln1_b"][i]))
        setb(f"g2{i}", _dup(p["ln2_g"][i]))
        setb(f"h2{i}", _dup(p["ln2_b"][i]))
        setb(f"gB{i}", _dup(p["blk_g"][i]))
        setb(f"hB{i}", _dup(p["blk_b"][i]))
    return W, B


def _split_waits(nc, mybir):
    """walrus here encodes at most ONE semaphore wait per instruction; move
    extras onto engine-matched NoOps inserted right before the instruction."""
    n = 0
    for f in nc.m.functions:
        for blk in f.blocks:
            out = []
            changed = False
            for inst in blk.instructions:
                si = inst.sync_info
                waits = list(si.on_wait) if si is not None and si.on_wait else []
                if len(waits) > 1:
                    for w in waits[:-1]:
                        nop = mybir.InstNoOp(
                            name=f"{inst.name}-wsplit{n}", ins=[], outs=[])
                        nop.engine = inst.engine
                        nop.sync_info = mybir.SyncInfo(on_wait=[w], on_update=[])
                        out.append(nop)
                        n += 1
                    si.on_wait = [waits[-1]]
                    inst.sync_info = si
                    changed = True
                out.append(inst)
            if changed:
                blk.instructions = out
    return n


def _build(flags):
    import contextlib
    import concourse.bass as bass
    import concourse.mybir as mybir
    import concourse.tile as tile

    ln1_full, ln2t, blkt, ln1t = flags
    wslots, bslots, wi, bi = _slot_layout(flags)
    NW = len(wslots)
    NB = len(bslots)

    F32 = mybir.dt.float32
    F32R = mybir.dt.float32r
    I32 = mybir.dt.int32
    AF = mybir.ActivationFunctionType
    Alu = mybir.AluOpType

    nc = bass.Bass("TRN2", target_bir_lowering=False, debug=False,
                   num_devices=NCORES)
    d_x = nc.dram_tensor("x_packed", [33, G], F32R, kind="ExternalInput")
    d_w = nc.dram_tensor("wstack", [128, NW * 128], F32R, kind="ExternalInput")
    d_b = nc.dram_tensor("bstack", [128, NB], F32, kind="ExternalInput")
    d_y = nc.dram_tensor("y_out", [24, G], F32, kind="ExternalOutput")
    d_xe = nc.dram_tensor("xe_out", [128, G], F32, kind="ExternalOutput")

    with tile.TileContext(nc) as tc, contextlib.ExitStack() as ctx:
        wpool = ctx.enter_context(tc.tile_pool(name="wpool", bufs=1))
        io = ctx.enter_context(tc.tile_pool(name="io", bufs=3))
        t2p = ctx.enter_context(tc.tile_pool(name="t2p", bufs=2))
        t3p = ctx.enter_context(tc.tile_pool(name="t3p", bufs=CFG["t3"]))
        t5p = ctx.enter_context(tc.tile_pool(name="t5p", bufs=CFG["t5"]))
        qp = ctx.enter_context(tc.tile_pool(name="qp", bufs=2))
        ps = ctx.enter_context(tc.tile_pool(name="ps", bufs=2, space="PSUM"))

        WS = wpool.tile([128, NW * 128], F32R)
        BS = wpool.tile([128, NB], F32)
        nc.sync.dma_start(out=BS, in_=d_b.ap())
        # globals + layer-0 weights first so the embed/layer-0 pipeline can
        # start while the rest of the weight stack streams in
        w_split = 24 * 128
        nc.sync.dma_start(out=WS[:, :w_split], in_=d_w.ap()[:, :w_split])
        nc.sync.dma_start(out=WS[:, w_split:], in_=d_w.ap()[:, w_split:])

        def w(name, kslice=None, mslice=None):
            k = wi[name]
            a = WS[:, k * 128:(k + 1) * 128]
            if kslice is not None:
                a = a[kslice[0]:kslice[1]]
            if mslice is not None:
                a = a[:, 0:mslice]
            return a

        def b(name, parts=128):
            return BS[0:parts, bi[name]:bi[name] + 1]

        def mm(out, name, rhs, start, stop, kslice=None, mslice=None,
               tile_position=None):
            nc.tensor.matmul(out, w(name, kslice, mslice), rhs,
                             start=start, stop=stop,
                             tile_position=tile_position)

        def relu_ps(out_ap, ps_ap, biascol, eng):
            if eng == "act":
                nc.scalar.activation(out_ap, ps_ap, AF.Relu,
                                     bias=b(biascol), scale=1.0)
            else:
                nc.vector.tensor_scalar(
                    out=out_ap, in0=ps_ap, scalar1=b(biascol), scalar2=0.0,
                    op0=Alu.add, op1=Alu.max)

        def evac_ps(out_ap, ps_ap, biascol, eng):
            if eng == "act":
                if biascol is None:
                    nc.scalar.copy(out_ap, ps_ap)
                else:
                    nc.scalar.activation(out_ap, ps_ap, AF.Identity,
                                         bias=b(biascol), scale=1.0)
            else:
                if biascol is None:
                    nc.vector.tensor_copy(out_ap, ps_ap)
                else:
                    nc.vector.tensor_scalar(
                        out=out_ap, in0=ps_ap, scalar1=b(biascol),
                        scalar2=None, op0=Alu.add)

        FCS = [CHUNKS[c] for c in range(len(CHUNKS))]

        def ln_stats(zp, Fc, cbcol, quadv, k):
            """Evacuate centered zp, square, reduce -> variance row-pair k of
            quadv. Returns z0s (SBUF, centered values incl. bias)."""
            z0s = t5p.tile([128, 512], F32, tag="z0s")
            evac_ps(z0s[:, :Fc], zp[:, :Fc], cbcol, CFG["e_z0s"])
            sq = t3p.tile([128, 512], F32R, tag="sq")
            if CFG["e_sq"] == "act":
                nc.scalar.activation(
                    sq[:, :Fc], zp[:, :Fc], AF.Square,
                    bias=(b(cbcol) if cbcol is not None else 0.0), scale=1.0)
            else:
                nc.gpsimd.tensor_tensor(out=sq[:, :Fc], in0=z0s[:, :Fc],
                                        in1=z0s[:, :Fc], op=Alu.mult)
            st = ps.tile([2, 512], F32, tag="stat", bufs=CFG["stat"])
            nc.tensor.matmul(st[:, :Fc], w("SBD", mslice=2), sq[:, :Fc],
                             start=True, stop=True)
            if k == 0:
                evac_ps(quadv[0:2, :Fc], st[:, :Fc], None, CFG["e_stat"])
            else:
                vt = t3p.tile([2, 512], F32, tag="vt")
                evac_ps(vt[:, :Fc], st[:, :Fc], None, CFG["e_stat"])
                nc.sync.dma_start(out=quadv[32 * k:32 * k + 2, :Fc],
                                  in_=vt[:, :Fc])
            return z0s

        def ln_rsqrt(quadv, top):
            """Shared seed + Newton over quad rows; returns y tile (F32R)."""
            seed = qp.tile([128, 512], I32, tag="qs")
            nc.vector.tensor_scalar(
                out=seed[0:top], in0=quadv[0:top].bitcast(I32),
                scalar1=-0.5, scalar2=MAGIC, op0=Alu.mult, op1=Alu.add)
            y = seed.bitcast(F32)
            for it in range(NR_ITERS):
                last = it == NR_ITERS - 1
                a1 = qp.tile([128, 512], F32, tag="qa")
                nc.vector.tensor_tensor(out=a1[0:top], in0=y[0:top],
                                        in1=y[0:top], op=Alu.mult)
                b1 = qp.tile([128, 512], F32, tag="qb")
                nc.vector.scalar_tensor_tensor(
                    out=b1[0:top], in0=a1[0:top], scalar=-0.5,
                    in1=quadv[0:top], op0=Alu.mult, op1=Alu.mult)
                y2 = qp.tile([128, 512], F32R if last else F32, tag="qy")
                nc.vector.scalar_tensor_tensor(
                    out=y2[0:top], in0=b1[0:top], scalar=1.5,
                    in1=y[0:top], op0=Alu.add, op1=Alu.mult)
                y = y2
            return y

        def ln_apply(z0s, yq, k, Fc, out_tile, gcol, hcol, trivial_affine):
            rb = ps.tile([128, 512], F32, tag="bc", bufs=CFG["bc"])
            tp = (96, 0) if k == 3 else None
            nc.tensor.matmul(rb[:, :Fc],
                             w("BBD")[32 * k:32 * k + 2],
                             yq[32 * k:32 * k + 2, :Fc],
                             start=True, stop=True, tile_position=tp)
            if trivial_affine:
                nc.vector.tensor_tensor(out=out_tile[:, :Fc],
                                        in0=z0s[:, :Fc], in1=rb[:, :Fc],
                                        op=Alu.mult)
            else:
                pre = t3p.tile([128, 512], F32, tag="lnpre")
                nc.vector.tensor_tensor(out=pre[:, :Fc], in0=z0s[:, :Fc],
                                        in1=rb[:, :Fc], op=Alu.mult)
                nc.vector.tensor_scalar(
                    out=out_tile[:, :Fc], in0=pre[:, :Fc],
                    scalar1=b(gcol), scalar2=b(hcol),
                    op0=Alu.mult, op1=Alu.add)

        # ================= main loop: groups of chunks =================
        for group in GROUPS:
            top = 32 * (len(group) - 1) + 2
            st = {}
            # ---- embed ----
            for k, c in enumerate(group):
                Fc = FCS[c]
                sl = slice(sum(FCS[:c]), sum(FCS[:c]) + Fc)
                xt = io.tile([33, 512], F32R, tag="xt")
                nc.sync.dma_start(out=xt[:, :Fc], in_=d_x.ap()[:, sl])
                ep = ps.tile([128, 512], F32, tag="acc", bufs=CFG["acc"])
                mm(ep[:, :Fc], "EMB", xt[:, :Fc], True, True, kslice=(0, 33))
                h = t5p.tile([128, 512], F32R, tag="h", bufs=CFG["h"])
                nc.scalar.copy(h[:, :Fc], ep[:, :Fc])
                nc.sync.dma_start(out=d_xe.ap()[:, sl],
                                  in_=h[:, :Fc].bitcast(F32))
                st[c] = {"h": h, "sl": sl, "Fc": Fc}

            for i in range(L):
                # ---- LN1 (full only when needed) ----
                if ln1_full[i]:
                    qv1 = qp.tile([128, 512], F32, tag="qv")
                    for k, c in enumerate(group):
                        Fc = st[c]["Fc"]
                        z1 = ps.tile([128, 512], F32, tag="acc", bufs=CFG["acc"])
                        mm(z1[:, :Fc], "CBD", st[c]["h"][:, :Fc], True, True)
                        st[c]["z1s"] = ln_stats(z1, Fc, None, qv1, len(group) - 1 - k)
                    yq1 = ln_rsqrt(qv1, top)
                    for k, c in enumerate(group):
                        Fc = st[c]["Fc"]
                        xn = t5p.tile([128, 512], F32R, tag="xn")
                        ln_apply(st[c]["z1s"], yq1, len(group) - 1 - k, Fc, xn,
                                 f"g1{i}", f"h1{i}", ln1t[i])
                        st[c]["xn"] = xn
                else:
                    for c in group:
                        st[c]["xn"] = st[c]["h"]

                # ---- front: tc1, cheb, tc2, ff -> LN2 stats ----
                qv2 = qp.tile([128, 512], F32, tag="qv")
                for k, c in enumerate(group):
                    Fc = st[c]["Fc"]
                    q = st[c]["h"]
                    xn = st[c]["xn"]
                    Pp = ps.tile([128, 512], F32, tag="mm", bufs=CFG["mm"])
                    mm(Pp[:, :Fc], f"P{i}", q[:, :Fc], True, True)
                    Qp = ps.tile([128, 512], F32, tag="mm", bufs=CFG["mm"])
                    mm(Qp[:, :Fc], f"Q{i}", q[:, :Fc], True, True)
                    sQ = t3p.tile([128, 512], F32, tag="sig")
                    nc.scalar.activation(sQ[:, :Fc], Qp[:, :Fc], AF.Sigmoid,
                                         bias=b(f"bQ{i}"), scale=1.0)
                    u = t3p.tile([128, 512], F32R, tag="u")
                    nc.vector.scalar_tensor_tensor(
                        out=u[:, :Fc], in0=Pp[:, :Fc], scalar=b(f"bP{i}"),
                        in1=sQ[:, :Fc], op0=Alu.add, op1=Alu.mult)
                    Rp = ps.tile([128, 512], F32, tag="mm", bufs=CFG["mm"])
                    mm(Rp[:, :Fc], f"R{i}", q[:, :Fc], True, False)
                    mm(Rp[:, :Fc], "IBD", u[:, :Fc], False, True)
                    t = t3p.tile([128, 512], F32R, tag="t")
                    relu_ps(t[:, :Fc], Rp[:, :Fc], f"bR{i}", CFG["e_t"])

                    Cp = ps.tile([128, 512], F32, tag="mm", bufs=CFG["mm"])
                    mm(Cp[:, :Fc], f"CH{i}", t[:, :Fc], True, True)
                    tt = t3p.tile([128, 512], F32R, tag="t")
                    relu_ps(tt[:, :Fc], Cp[:, :Fc], f"bC{i}", CFG["e_t2"])

                    P2p = ps.tile([128, 512], F32, tag="mm", bufs=CFG["mm"])
                    mm(P2p[:, :Fc], f"P2{i}", tt[:, :Fc], True, True)
                    Q2p = ps.tile([128, 512], F32, tag="mm", bufs=CFG["mm"])
                    mm(Q2p[:, :Fc], f"Q2{i}", tt[:, :Fc], True, True)
                    sQ2 = t3p.tile([128, 512], F32, tag="sig")
                    nc.scalar.activation(sQ2[:, :Fc], Q2p[:, :Fc], AF.Sigmoid,
                                         bias=b(f"bQ2{i}"), scale=1.0)
                    u2 = t3p.tile([128, 512], F32R, tag="u")
                    nc.vector.scalar_tensor_tensor(
                        out=u2[:, :Fc], in0=P2p[:, :Fc], scalar=b(f"bP2{i}"),
                        in1=sQ2[:, :Fc], op0=Alu.add, op1=Alu.mult)
                    R2p = ps.tile([128, 512], F32, tag="mm", bufs=CFG["mm"])
                    mm(R2p[:, :Fc], f"R2{i}", tt[:, :Fc], True, False)
                    mm(R2p[:, :Fc], "IBD", u2[:, :Fc], False, True)
                    o = t5p.tile([128, 512], F32R, tag="o")
                    relu_ps(o[:, :Fc], R2p[:, :Fc], f"bR2{i}", CFG["e_o"])
                    st[c]["o"] = o

                    zp = ps.tile([128, 512], F32, tag="acc", bufs=CFG["acc"])
                    mm(zp[:, :Fc], "CBD", xn[:, :Fc], True, False)
                    for j in range(4):
                        fp = ps.tile([128, 512], F32, tag="mm", bufs=CFG["mm"])
                        mm(fp[:, :Fc], f"F1{i}_{j}", xn[:, :Fc], True, True)
                        fj = t3p.tile([128, 512], F32R, tag="fj")
                        relu_ps(fj[:, :Fc], fp[:, :Fc], f"b1{i}_{j}",
                                "act" if j < CFG["ff_act"] else "dve")
                        mm(zp[:, :Fc], f"F2{i}_{j}", fj[:, :Fc],
                           False, j == 3)
                    st[c]["z2s"] = ln_stats(zp, Fc, f"cb2{i}", qv2, len(group) - 1 - k)

                yq2 = ln_rsqrt(qv2, top)

                # ---- mid: us, gating, blk stats ----
                qvb = qp.tile([128, 512], F32, tag="qv")
                for k, c in enumerate(group):
                    Fc = st[c]["Fc"]
                    q = st[c]["h"]
                    o = st[c]["o"]
                    us = t3p.tile([128, 512], F32R, tag="us")
                    ln_apply(st[c]["z2s"], yq2, len(group) - 1 - k, Fc, us,
                             f"g2{i}", f"h2{i}", ln2t[i])
                    fgp = ps.tile([128, 512], F32, tag="mm", bufs=CFG["mm"])
                    mm(fgp[:, :Fc], f"FGQ{i}", q[:, :Fc], True, False)
                    mm(fgp[:, :Fc], f"FGO{i}", o[:, :Fc], False, True)
                    fgx = t3p.tile([128, 512], F32R, tag="fgx")
                    evac_ps(fgx[:, :Fc], fgp[:, :Fc], f"bFG{i}", CFG["e_fgx"])
                    fsp = ps.tile([128, 512], F32, tag="mm", bufs=CFG["mm"])
                    mm(fsp[:, :Fc], "IBD", fgx[:, :Fc], True, False)
                    mm(fsp[:, :Fc], f"FS{i}", us[:, :Fc], False, True)
                    g = t3p.tile([128, 512], F32, tag="g")
                    nc.scalar.activation(g[:, :Fc], fsp[:, :Fc], AF.Sigmoid,
                                         bias=b(f"bFS{i}"), scale=1.0)
                    dd = t3p.tile([128, 512], F32, tag="e")
                    nc.gpsimd.tensor_tensor(out=dd[:, :Fc],
                                            in0=us[:, :Fc].bitcast(F32),
                                            in1=fgx[:, :Fc].bitcast(F32),
                                            op=Alu.subtract)
                    e = t3p.tile([128, 512], F32R, tag="u")
                    nc.vector.tensor_tensor(out=e[:, :Fc], in0=g[:, :Fc],
                                            in1=dd[:, :Fc], op=Alu.mult)
                    zb = ps.tile([128, 512], F32, tag="acc", bufs=CFG["acc"])
                    mm(zb[:, :Fc], "CBD", fgx[:, :Fc], True, False)
                    mm(zb[:, :Fc], "CBD", q[:, :Fc], False, False)
                    mm(zb[:, :Fc], "CBD", e[:, :Fc], False, True)
                    st[c]["zbs"] = ln_stats(zb, Fc, None, qvb, len(group) - 1 - k)

                yqb = ln_rsqrt(qvb, top)

                # ---- tail: normalize into next h ----
                for k, c in enumerate(group):
                    Fc = st[c]["Fc"]
                    hn = t5p.tile([128, 512], F32R, tag="h", bufs=CFG["h"])
                    ln_apply(st[c]["zbs"], yqb, len(group) - 1 - k, Fc, hn,
                             f"gB{i}", f"hB{i}", blkt[i])
                    st[c]["h"] = hn

            # ---- output projection ----
            for k, c in enumerate(group):
                Fc = st[c]["Fc"]
                yp = ps.tile([24, 512], F32, tag="acc", bufs=CFG["acc"])
                mm(yp[:, :Fc], "OUT", st[c]["h"][:, :Fc], True, True,
                   mslice=24)
                yt = io.tile([24, 512], F32, tag="yt")
                nc.scalar.activation(yt[:, :Fc], yp[:, :Fc], AF.Identity,
                                     bias=b("BOUT", 24), scale=1.0)
                nc.sync.dma_start(out=d_y.ap()[:, st[c]["sl"]],
                                  in_=yt[:, :Fc])

    _split_waits(nc, mybir)
    return nc


def _get_compiled(flags):
    if flags not in _COMPILED:
        _COMPILED[flags] = _build(flags)
    return _COMPILED[flags]


def kernel(x, edge_index, edge_weight, params):
    from concourse import bass_utils

    x = np.asarray(x, dtype=np.float32)
    p = {k: np.asarray(v, dtype=np.float32) for k, v in params.items()}

    flags = _make_flags(p)
    nc = _get_compiled(flags)

    W, B = _pack_host(p, flags)

    xpad = np.zeros((NPAD, F_IN), np.float32)
    xpad[:N] = x
    xc = xpad.reshape(NCORES, CN, F_IN)
    in_maps = []
    for c in range(NCORES):
        xd = np.ones((33, G), np.float32)
        xd[0:16] = xc[c, :G].T
        xd[16:32] = xc[c, G:].T
        in_maps.append({"x_packed": xd, "wstack": W, "bstack": B})

    res = bass_utils.run_bass_kernel_spmd(
        nc, in_maps, core_ids=list(range(NCORES)), trace=TRACE)
    global LAST_EXEC_NS, LAST_RESULTS
    LAST_EXEC_NS = res.exec_time_ns
    LAST_RESULTS = res

    out = np.zeros((NPAD, OUT_LEN), np.float32)
    xe = np.zeros((NPAD, D), np.float32)
    for c in range(NCORES):
        y = res.results[c]["y_out"]
        e = res.results[c]["xe_out"]
        base = c * CN
        out[base:base + G] = y[0:12].T
        out[base + G:base + CN] = y[12:24].T
        xe[base:base + G] = e[0:64].T
        xe[base + G:base + CN] = e[64:128].T
    return out[:N], xe[:N]
